# revision 1
# baseline (speedup 1.0000x reference)
"""Mesh vertex-normals kernel v3 for 8 TRN2 NeuronCores (Bass/Tile).

Structure (per core, on its row band of the padded vertex grid):
  * SoA layout: tiles are [rows, 3, cols] fp32 planes, so every
    elementwise op (including the 12 cross-product component mults) is
    unit-stride on the free axis.
  * Folded band: the 58-row leftover block is folded into column strips
    stacked on the partition axis (116 busy partitions instead of 58).
  * S-path (edges, cross products, T=C1+C2) computed in f32: any fp16
    rounding before the stencil sum blows up the ~70 vertices whose
    aggregate normal nearly cancels (|S| ~ 0.02) past the 2e-2 gate.
  * The vertex-normal stencil S = SH@t(c+1) + SH@c1 + I@t + I@c2(c+1)
    accumulates on the tensor engine into PSUM (SH = shift-down-one-
    partition matrix), eliminating the p/q/s adds from the vector
    engines.
  * Norm tail in fp16 (safe: rounding the final S is relative error):
    ACT Square from PSUM -> nsq adds -> ACT Sqrt(+eps) -> reciprocal ->
    packed fp16 multiply; output stored as fp16 planes, host converts.
  * Engine split is DVE-heavy: gpsimd (Pool) measures ~2.2 ns/elem on
    real HW vs DVE 1.04 (the v1 sim model's 0.833 for Pool is wrong).
  * Emission is software-pipelined: stage k of unit u emits at step
    u + k, so each in-order engine queue interleaves independent units
    (~2.6x faster than unit-sequential emission on HW).

Host side: pad (edge mode) + transpose to [rows, 3, cols] f32 planes;
output fp16 planes -> f32 [N, 3].
"""

import sys

sys.path.insert(0, "/opt/trn_rl_repo")

import numpy as np

GRID = 1449
N_CORES = 8


# ---------------------------------------------------------------------------
# host-side helpers
# ---------------------------------------------------------------------------

def _is_structured(faces: np.ndarray, grid: int) -> bool:
    n_quads = (grid - 1) * (grid - 1)
    if faces.shape != (2 * n_quads, 3):
        return False
    idx = np.arange(grid * grid, dtype=np.int64).reshape(grid, grid)
    i00 = idx[:-1, :-1].ravel()
    i01 = idx[:-1, 1:].ravel()
    i10 = idx[1:, :-1].ravel()
    i11 = idx[1:, 1:].ravel()
    f = faces
    return (
        np.array_equal(f[:n_quads, 0], i00)
        and np.array_equal(f[:n_quads, 1], i01)
        and np.array_equal(f[:n_quads, 2], i11)
        and np.array_equal(f[n_quads:, 0], i00)
        and np.array_equal(f[n_quads:, 1], i11)
        and np.array_equal(f[n_quads:, 2], i10)
    )


def _host_fallback(vertices: np.ndarray, faces: np.ndarray) -> np.ndarray:
    n_vertices = vertices.shape[0]
    va = vertices[faces[:, 0]]
    vb = vertices[faces[:, 1]]
    vc = vertices[faces[:, 2]]
    cross = np.cross(vb - va, vc - vb).astype(np.float32)
    norm = np.linalg.norm(cross, axis=-1, keepdims=True)
    weighted = (cross / norm) * (norm * 0.5)
    data = np.broadcast_to(weighted[:, None, :], (faces.shape[0], 3, 3)).reshape(-1, 3)
    summed = np.zeros((n_vertices, 3), dtype=np.float32)
    np.add.at(summed, faces.reshape(-1), data)
    norms = np.linalg.norm(summed, axis=-1, keepdims=True)
    return (summed / np.maximum(norms, 1e-10)).astype(np.float32)


def _band_layout(grid: int, n_cores: int):
    base = (grid - 1) // n_cores
    assert base * n_cores == grid - 1, "grid-1 must divide evenly"
    out_rows = base + 1
    in_rows = base + 3
    return base, out_rows, in_rows


def _col_chunks(width: int, chunk: int):
    return [(c0, min(chunk, width - c0)) for c0 in range(0, width, chunk)]


def _overlap_chunks(total: int, n: int):
    """n equal-width chunks covering [0, total); later chunks may overlap
    earlier ones. Yields (c0, so, wst): load cols c0..c0+w, store local
    cols so..so+wst to grid cols c0+so..c0+so+wst. All widths equal w."""
    w = -(-total // n)
    out = []
    for j in range(n):
        store_start = j * w
        store_end = min((j + 1) * w, total)
        c0 = min(j * w, total - w)
        out.append((c0, store_start - c0, store_end - store_start))
    return w, out


def _fold_units(grid: int, n_cores: int, chunks_a: int, chunks_b: int = 1):
    """Units: each = dict(P, w, rects=[(p0, nv, r0, c0, so, wst)]).

    Rect semantics: partitions p0..p0+nv hold padded-band v-rows
    r0..r0+nv; loads fetch w+2 cols from c0; stores write local cols
    so..so+wst to grid cols c0+so..
    """
    base, out_rows, in_rows = _band_layout(grid, n_cores)
    units = []
    if in_rows <= 128:
        w, chunks = _overlap_chunks(grid, chunks_a)
        for c0, so, wst in chunks:
            units.append(dict(P=in_rows, w=w,
                              rects=[(0, in_rows, 0, c0, so, wst)]))
        return units
    # 128-row rect A + leftover rect B folded into column strips
    nv_b = in_rows - 126
    assert nv_b >= 3
    w, chunks = _overlap_chunks(grid, chunks_a)
    for c0, so, wst in chunks:
        units.append(dict(P=128, w=w, rects=[(0, 128, 0, c0, so, wst)]))
    nstrips = 128 // nv_b
    wb, bstrips = _overlap_chunks(grid, nstrips)
    # chunk each strip's columns as well, so B tiles stay small
    wbc, bcols = _overlap_chunks(wb, chunks_b)
    for (coff, so2, wst2) in bcols:
        ch_lo, ch_hi = coff + so2, coff + so2 + wst2
        rects = []
        for j, (c0s, so, wst) in enumerate(bstrips):
            # intersect the strip's store range [so, so+wst) with the
            # column chunk's store range, both in strip-local coords
            lo = max(so, ch_lo)
            hi = min(so + wst, ch_hi)
            if hi <= lo:
                continue
            rects.append((j * nv_b, nv_b, 126, c0s + coff, lo - coff, hi - lo))
        units.append(dict(P=nstrips * nv_b, w=wbc, rects=rects))
    return units


# ---------------------------------------------------------------------------
# device program
# ---------------------------------------------------------------------------

DEFAULT_CFG = dict(
    chunks_a=5,
    chunks_b=3,
    sw_pipe=1,
    io_bufs=3,
    wk_bufs=4,
    psum_bufs=4,
    psum_cols=150,     # cols per PSUM chunk (x3 planes x4B <= 2KB)
    sq="act",          # 'act' | 'v' | 'g'
    o_bcast=True,
    o_s16=True,        # ACT-copy S from PSUM to fp16, packed multiply
    vyf_pe=False,      # vdn double-load + DVE subtract (ACT is precious)
    pshift="pe",
    stencil="pe",      # S = SH@t(c+1) + SH@c1 + I@t + I@c2(c+1) in PSUM
    # engine per op: 'v' = vector (DVE), 'g' = gpsimd (Pool).
    # Pool measures ~2.2 ns/elem on HW vs DVE 1.04, so DVE-heavy.
    eng=dict(vyf="v", hx="v", dd="v", mm1="v", mm2="g", c1="g", c2="g",
             t="v", p="g", q="v", s="v", nsq="v", o="v"),
    mm_map="vgvgvgvgvgvv",
    st_eng="act",
    s_f16=True,
)


def _cfg_key(cfg):
    e = cfg["eng"]
    return (cfg["chunks_a"], cfg.get("chunks_b", 1),
            cfg["io_bufs"], cfg["wk_bufs"], cfg["sq"],
            cfg.get("o_bcast", True), cfg.get("vyf_pe", False),
            cfg.get("pshift", "dma"), cfg.get("psum_cols", 162),
            cfg.get("psum_bufs", 4), cfg.get("st_eng", "act"),
            cfg.get("s_f16", True), cfg.get("stencil", "dve"),
            cfg.get("mm_map"), cfg.get("o_s16", False),
            cfg.get("nsq_pe", False), cfg.get("fine_stages", False),
            cfg.get("norm_chunked", False),
            tuple(sorted(e.items())))


def _build_program(grid: int, n_cores: int, repeats: int = 1, cfg=None):
    import contextlib

    import concourse.bacc as bacc
    import concourse.tile as tile
    from concourse import mybir

    cfg = cfg or DEFAULT_CFG
    f16 = mybir.dt.float16
    f32 = mybir.dt.float32

    base, out_rows, in_rows = _band_layout(grid, n_cores)
    W = grid + 2

    nc = bacc.Bacc()
    vband = nc.dram_tensor("vband", [in_rows, 3, W], f32, kind="ExternalInput")
    oband = nc.dram_tensor("oband", [out_rows, 3, grid], f16,
                           kind="ExternalOutput")

    units = _fold_units(grid, n_cores, cfg["chunks_a"], cfg.get("chunks_b", 1))
    for i, u in enumerate(units):
        u["idx"] = i

    with tile.TileContext(nc) as tc:
        with (
            tc.tile_pool(name="io", bufs=cfg["io_bufs"]) as io,
            tc.tile_pool(name="wk", bufs=cfg["wk_bufs"]) as wk,
            tc.tile_pool(name="ps", bufs=cfg.get("psum_bufs", 4),
                         space="PSUM") as psp,
            tc.tile_pool(name="cst", bufs=1) as cst,
        ):
            from concourse.masks import make_identity

            eps_tile = cst.tile([128, 1], f16, tag="eps")
            nc.vector.memset(eps_tile[:, :], 1e-7)
            # tid[:, 1:129] = down-shift matrix SH[k, m] = 1 iff k == m+1
            tid = cst.tile([128, 130], f32, tag="tid")
            nc.gpsimd.memset(tid[:, :], 0.0)
            make_identity(nc, tid[:, 0:128], nomemset=True)
            tid16 = None
            if cfg.get("nsq_pe", False):
                tid16 = cst.tile([128, 128], f16, tag="tid16")
                nc.gpsimd.memset(tid16[:, :], 0.0)
                make_identity(nc, tid16[:, :], nomemset=True)
            # tmix[:, 1:129][k, m] = +1 if k == m+1 else (-1 if k == m)
            tmix = cst.tile([128, 130], f32, tag="tmix")
            nc.gpsimd.memset(tmix[:, :], 0.0)
            make_identity(nc, tmix[:, 0:128], nomemset=True)
            nc.gpsimd.affine_select(
                out=tmix[:, 1:129], in_=tmix[:, 1:129],
                compare_op=mybir.AluOpType.not_equal, fill=-1.0, base=0,
                pattern=[[-1, 128]], channel_multiplier=1,
            )

            loop = tc.For_i(0, repeats, 1) if repeats > 1 else contextlib.nullcontext()
            with loop:
                stages = [
                    _emit_unit(nc, io, wk, psp, eps_tile, tid, tmix, unit,
                               vband, oband, mybir, cfg, tid16)
                    for unit in units
                ]
                skew = cfg.get("sw_pipe", 0)
                if skew:
                    # software pipeline: stage k of unit u emits at step
                    # u + k*skew; later stages (older units) first, so each
                    # engine's queue interleaves independent units.
                    nst = len(stages[0])
                    total = len(units) + (nst - 1) * skew
                    for step in range(total):
                        for stg in range(nst - 1, -1, -1):
                            ui = step - stg * skew
                            if 0 <= ui < len(units):
                                stages[ui][stg]()
                else:
                    for fs in stages:
                        for f in fs:
                            f()

    nc.finalize()
    return nc


def _psum_chunks(width: int, chunk: int):
    return [(j0, min(chunk, width - j0)) for j0 in range(0, width, chunk)]


def _emit_unit(nc, io, wk, psp, eps_tile, tid, tmix, unit, vband, oband,
               mybir, cfg, tid16=None):
    """Returns a list of stage closures: [load, vyf, crosses, stencil, norm].

    Calling them in order emits the unit; a software-pipelined caller can
    interleave stages of different units.
    """
    f16 = mybir.dt.float16
    f32 = mybir.dt.float32
    Alu = mybir.AluOpType
    Act = mybir.ActivationFunctionType
    ui = unit.get("idx", 0)
    ENG = {"v": nc.vector, "g": nc.gpsimd,
           "a": nc.vector if ui % 2 == 0 else nc.gpsimd,
           "b": nc.gpsimd if ui % 2 == 0 else nc.vector}
    eng = {k: ENG[v] for k, v in cfg["eng"].items()}

    def tt(tag, out, in0, in1, op):
        eng[tag].tensor_tensor(out=out, in0=in0, in1=in1, op=op)

    P, w, rects = unit["P"], unit["w"], unit["rects"]
    w2 = w + 2
    in_rows = vband.shape[0]
    pcols = cfg.get("psum_cols", 162)
    ts = {}  # tiles shared across stages

    def stage_load():
        v = ts["v"] = io.tile([P, 3, w2], f32, tag="v", name="v")
        for (p0, nv, r0, c0, so, wst) in rects:
            nc.sync.dma_start(out=v[p0:p0 + nv, :, :],
                              in_=vband[r0:r0 + nv, :, c0:c0 + w2])
        if not cfg.get("vyf_pe", False):
            vdn = ts["vdn"] = io.tile([P, 3, w2], f32, tag="vdn", name="vdn")
            for (p0, nv, r0, c0, so, wst) in rects:
                # duplicate the band's last row if the shifted window runs
                # off the end (that partition is never used)
                n_load = min(nv, in_rows - (r0 + 1))
                nc.sync.dma_start(
                    out=vdn[p0:p0 + n_load, :, :],
                    in_=vband[r0 + 1:r0 + 1 + n_load, :, c0:c0 + w2])
                if n_load < nv:
                    nc.sync.dma_start(
                        out=vdn[p0 + nv - 1:p0 + nv, :, :],
                        in_=vband[in_rows - 1:in_rows, :, c0:c0 + w2])

    def stage_vyf():
        v = ts["v"]
        vyf = ts["vyf"] = wk.tile([P, 3, w2], f32, tag="vyf", name="vyf")
        if cfg.get("vyf_pe", False):
            # vyf = (SH - I) @ v on the tensor engine; ACT copies PSUM out.
            # Seam partitions mix adjacent rects; they are never consumed.
            for j0, pw in _psum_chunks(w2, pcols):
                psv = psp.tile([128, 3, pw], f32, tag="psv", name="psv")
                nc.tensor.matmul(out=psv[:, :, :], lhsT=tmix[0:P, 1:129],
                                 rhs=v[:, :, j0:j0 + pw], start=True, stop=True)
                nc.scalar.activation(out=vyf[:, :, j0:j0 + pw],
                                     in_=psv[0:P, :, :], func=Act.Copy)
        else:
            tt("vyf", vyf[:, :, :], ts["vdn"][:, :, :], v[:, :, :],
               Alu.subtract)
        hx = ts["hx"] = wk.tile([P, 3, w + 1], f32, tag="hx", name="hx")
        tt("hx", hx[:, :, :], v[:, :, 1:w2], v[:, :, 0:w + 1], Alu.subtract)

    mm_map = cfg.get("mm_map")

    def mm(idx, dflt, out_, a, b):
        e = ENG[mm_map[idx]] if mm_map else eng[dflt]
        e.tensor_tensor(out=out_, in0=a, in1=b, op=Alu.mult)

    def stage_cross_a():
        vyf, hx = ts["vyf"], ts["hx"]
        dd = ts["dd"] = wk.tile([P, 3, w + 1], f32, tag="dd", name="dd")
        if "vdn" in ts:
            # dd = vdn(c+1) - v(c): straight from loads (one rounding,
            # no dependency on hx/vyf -> shorter critical chain)
            tt("dd", dd[:, :, :], ts["vdn"][:, :, 1:w2],
               ts["v"][:, :, 0:w + 1], Alu.subtract)
        else:
            tt("dd", dd[:, :, :], hx[:, :, :], vyf[:, :, 1:w2], Alu.add)
        m1 = wk.tile([P, 3, w + 1], f32, tag="m1", name="m1")
        m2 = wk.tile([P, 3, w + 1], f32, tag="m2", name="m2")
        c1 = ts["c1"] = wk.tile([P, 3, w + 1], f32, tag="c1", name="c1")
        for k in range(3):
            u, x = (k + 1) % 3, (k + 2) % 3
            mm(2 * k, "mm1", m1[:, k:k + 1, :], hx[:, u:u + 1, :],
               vyf[:, x:x + 1, 1:w2])
            mm(2 * k + 1, "mm2", m2[:, k:k + 1, :], hx[:, x:x + 1, :],
               vyf[:, u:u + 1, 1:w2])
        tt("c1", c1[:, :, :], m1[:, :, :], m2[:, :, :], Alu.subtract)

    def stage_cross_b():
        vyf, dd = ts["vyf"], ts["dd"]
        m3 = wk.tile([P, 3, w + 1], f32, tag="m1", name="m3")
        m4 = wk.tile([P, 3, w + 1], f32, tag="m2", name="m4")
        c2 = ts["c2"] = wk.tile([P, 3, w + 1], f32, tag="c2", name="c2")
        for k in range(3):
            u, x = (k + 1) % 3, (k + 2) % 3
            mm(6 + 2 * k, "mm1", m3[:, k:k + 1, :], dd[:, u:u + 1, :],
               vyf[:, x:x + 1, 0:w + 1])
            mm(7 + 2 * k, "mm2", m4[:, k:k + 1, :], dd[:, x:x + 1, :],
               vyf[:, u:u + 1, 0:w + 1])
        tt("c2", c2[:, :, :], m3[:, :, :], m4[:, :, :], Alu.subtract)

    def stage_cross():
        stage_cross_a()
        stage_cross_b()

    def stage_stencil():
        c1, c2 = ts["c1"], ts["c2"]
        # T = C1+C2; P = T(c+1)+C1; Q = T+C2(c+1); S = down(P)+Q
        t = wk.tile([P, 3, w + 1], f32, tag="t", name="t")
        tt("t", t[:, :, :], c1[:, :, :], c2[:, :, :], Alu.add)
        if cfg.get("stencil", "dve") == "pe":
            # S accumulates fully in PSUM:
            #   S = SH@t(c+1) + SH@c1(c) + I@t(c) + I@c2(c+1)
            ts["pss"] = []
            for j0, pw in _psum_chunks(w, pcols):
                pss = psp.tile([128, 3, pw], f32, tag="pss", name="pss")
                I, SH = tid[0:P, 0:128], tid[0:P, 1:129]
                nc.tensor.matmul(out=pss[:, :, :], lhsT=SH,
                                 rhs=t[:, :, 1 + j0:1 + j0 + pw],
                                 start=True, stop=False)
                nc.tensor.matmul(out=pss[:, :, :], lhsT=SH,
                                 rhs=c1[:, :, j0:j0 + pw],
                                 start=False, stop=False)
                nc.tensor.matmul(out=pss[:, :, :], lhsT=I,
                                 rhs=t[:, :, j0:j0 + pw],
                                 start=False, stop=False)
                nc.tensor.matmul(out=pss[:, :, :], lhsT=I,
                                 rhs=c2[:, :, 1 + j0:1 + j0 + pw],
                                 start=False, stop=True)
                ts["pss"].append((j0, pw, pss))
            ts["Q"] = P
            return
        p = wk.tile([P, 3, w], f32, tag="dd", name="p")
        tt("p", p[:, :, :], t[:, :, 1:w + 1], c1[:, :, 0:w], Alu.add)
        q = wk.tile([P, 3, w], f32, tag="q", name="q")
        tt("q", q[:, :, :], t[:, :, 0:w], c2[:, :, 1:w + 1], Alu.add)

        sdt = f16 if cfg.get("s_f16", True) else f32
        s = ts["s"] = wk.tile([P, 3, w], sdt, tag="hx", name="s")
        if cfg.get("pshift", "dma") == "pe":
            # s = SH @ p + q: the shift runs on the tensor engine into PSUM
            for j0, pw in _psum_chunks(w, pcols):
                pss = psp.tile([128, 3, pw], f32, tag="pss", name="pss")
                nc.tensor.matmul(out=pss[:, :, :], lhsT=tid[0:P, 1:129],
                                 rhs=p[:, :, j0:j0 + pw], start=True,
                                 stop=True)
                tt("s", s[:, :, j0:j0 + pw], pss[0:P, :, :],
                   q[:, :, j0:j0 + pw], Alu.add)
            ts["Q"] = P
        else:
            # full-tile partition shift; seam partitions get cross-rect
            # garbage, which post-shift ops compute on but stores never read
            pdn = wk.tile([P, 3, w], f32, tag="vyf", name="pdn")
            nc.sync.dma_start(out=pdn[0:P - 1, :, :], in_=p[1:P, :, :])
            ts["Q"] = P - 1
            tt("s", s[0:P - 1, :, :], pdn[0:P - 1, :, :], q[0:P - 1, :, :],
               Alu.add)

    def stage_norm_chunked():
        # per-PSUM-chunk norm tail: sq/nsq/rn/o column-local, so each
        # chunk finishes (and stores) without waiting for the other
        Q = ts["Q"]
        o = io.tile([P, 3, w], f16, tag="o", name="o")
        for j0, pw, pss in ts["pss"]:
            sq = wk.tile([P, 3, pw], f16, tag="m1", name="sq")
            nc.scalar.activation(out=sq[0:Q, :, :], in_=pss[0:Q, :, :],
                                 func=Act.Square)
            nsq = wk.tile([P, 1, pw], f16, tag="nsq", name="nsq")
            tt("nsq", nsq[0:Q, :, :], sq[0:Q, 0:1, :], sq[0:Q, 1:2, :],
               Alu.add)
            tt("nsq", nsq[0:Q, :, :], nsq[0:Q, :, :], sq[0:Q, 2:3, :],
               Alu.add)
            rn = wk.tile([P, 1, pw], f16, tag="rn", name="rn")
            nc.scalar.activation(out=rn[0:Q, :, :], in_=nsq[0:Q, :, :],
                                 func=Act.Sqrt, bias=eps_tile[:Q, :])
            with nc.allow_low_precision(reason="1/norm fine in fp16"):
                if cfg["eng"].get("rcp", "v") == "v":
                    nc.vector.reciprocal(out=rn[0:Q, :, :], in_=rn[0:Q, :, :])
                else:
                    nc.gpsimd.reciprocal(out=rn[0:Q, :, :], in_=rn[0:Q, :, :])
            s16 = wk.tile([P, 3, pw], f16, tag="m2", name="s16")
            nc.scalar.activation(out=s16[0:Q, :, :], in_=pss[0:Q, :, :],
                                 func=Act.Copy)
            tt("o", o[0:Q, :, j0:j0 + pw], s16[0:Q, :, :],
               rn[0:Q, :, :].broadcast_to((Q, 3, pw)), Alu.mult)
        st = {"sp": nc.sync, "act": nc.scalar,
              "g": nc.gpsimd}[cfg.get("st_eng", "act")]
        for (p0, nv, r0, c0, so, wst) in rects:
            ns = nv - 2
            st.dma_start(out=oband[r0:r0 + ns, :, c0 + so:c0 + so + wst],
                         in_=o[p0:p0 + ns, :, so:so + wst])

    def stage_norm():
        if cfg.get("norm_chunked", False) and cfg.get("stencil") == "pe":
            stage_norm_chunked()
            return
        Q = ts["Q"]
        sq = wk.tile([P, 3, w], f16, tag="m1", name="sq")
        if cfg.get("stencil", "dve") == "pe":
            # S lives in PSUM chunks; square from PSUM, and o multiplies
            # the PSUM value directly.
            for j0, pw, pss in ts["pss"]:
                nc.scalar.activation(out=sq[0:Q, :, j0:j0 + pw],
                                     in_=pss[0:Q, :, :], func=Act.Square)
        elif cfg["sq"] == "act":
            nc.scalar.activation(out=sq[0:Q, :, :], in_=ts["s"][0:Q, :, :],
                                 func=Act.Square)
        else:
            ENG[cfg["sq"]].tensor_tensor(out=sq[0:Q, :, :],
                                         in0=ts["s"][0:Q, :, :],
                                         in1=ts["s"][0:Q, :, :], op=Alu.mult)
        rn = wk.tile([P, 1, w], f16, tag="rn", name="rn")
        if cfg.get("nsq_pe", False):
            # nsq = sq_x + sq_y + sq_z as 3 fp16 identity matmuls in PSUM
            psn = psp.tile([128, 1, w], f32, tag="psn", name="psn")
            for k in range(3):
                nc.tensor.matmul(out=psn[:, :, :], lhsT=tid16[0:P, :],
                                 rhs=sq[:, k:k + 1, :], start=(k == 0),
                                 stop=(k == 2))
            nc.scalar.activation(out=rn[0:Q, :, :], in_=psn[0:Q, :, :],
                                 func=Act.Sqrt, bias=eps_tile[:Q, :])
        else:
            nsq = wk.tile([P, 1, w], f16, tag="nsq", name="nsq")
            tt("nsq", nsq[0:Q, :, :], sq[0:Q, 0:1, :], sq[0:Q, 1:2, :],
               Alu.add)
            tt("nsq", nsq[0:Q, :, :], nsq[0:Q, :, :], sq[0:Q, 2:3, :],
               Alu.add)
            nc.scalar.activation(out=rn[0:Q, :, :], in_=nsq[0:Q, :, :],
                                 func=Act.Sqrt, bias=eps_tile[:Q, :])
        with nc.allow_low_precision(reason="1/norm fine in fp16"):
            if cfg["eng"].get("rcp", "v") == "v":
                nc.vector.reciprocal(out=rn[0:Q, :, :], in_=rn[0:Q, :, :])
            else:
                nc.gpsimd.reciprocal(out=rn[0:Q, :, :], in_=rn[0:Q, :, :])
        o = io.tile([P, 3, w], f16, tag="o", name="o")
        if cfg.get("stencil", "dve") == "pe":
            if cfg.get("o_s16", False):
                s16 = wk.tile([P, 3, w], f16, tag="m2", name="s16")
                for j0, pw, pss in ts["pss"]:
                    nc.scalar.activation(out=s16[0:Q, :, j0:j0 + pw],
                                         in_=pss[0:Q, :, :], func=Act.Copy)
                tt("o", o[0:Q, :, :], s16[0:Q, :, :],
                   rn[0:Q, :, :].broadcast_to((Q, 3, w)), Alu.mult)
            else:
                for j0, pw, pss in ts["pss"]:
                    tt("o", o[0:Q, :, j0:j0 + pw], pss[0:Q, :, :],
                       rn[0:Q, :, j0:j0 + pw].broadcast_to((Q, 3, pw)),
                       Alu.mult)
        elif cfg.get("o_bcast", True):
            tt("o", o[0:Q, :, :], ts["s"][0:Q, :, :],
               rn[0:Q, :, :].broadcast_to((Q, 3, w)), Alu.mult)
        else:
            for k in range(3):
                tt("o", o[0:Q, k:k + 1, :], ts["s"][0:Q, k:k + 1, :],
                   rn[0:Q, :, :], Alu.mult)
        st = {"sp": nc.sync, "act": nc.scalar,
              "g": nc.gpsimd}[cfg.get("st_eng", "act")]
        for (p0, nv, r0, c0, so, wst) in rects:
            ns = nv - 2
            st.dma_start(out=oband[r0:r0 + ns, :, c0 + so:c0 + so + wst],
                         in_=o[p0:p0 + ns, :, so:so + wst])

    if cfg.get("fine_stages", False):
        return [stage_load, stage_vyf, stage_cross_a, stage_cross_b,
                stage_stencil, stage_norm]
    return [stage_load, stage_vyf, stage_cross, stage_stencil, stage_norm]


_PROGRAM_CACHE: dict = {}


def _get_program(grid: int, n_cores: int, repeats: int = 1, cfg=None):
    cfg = cfg or DEFAULT_CFG
    key = (grid, n_cores, repeats, _cfg_key(cfg))
    if key not in _PROGRAM_CACHE:
        _PROGRAM_CACHE[key] = _build_program(grid, n_cores, repeats, cfg)
    return _PROGRAM_CACHE[key]


def _make_in_maps(vertices: np.ndarray, grid: int, n_cores: int):
    base, out_rows, in_rows = _band_layout(grid, n_cores)
    V = vertices.reshape(grid, grid, 3)
    VP = np.pad(V, ((1, 1), (1, 1), (0, 0)), mode="edge")
    VPT = np.ascontiguousarray(VP.transpose(0, 2, 1))
    return [
        {"vband": np.ascontiguousarray(VPT[base * k: base * k + in_rows])}
        for k in range(n_cores)
    ]


def _assemble_out(results, grid: int, n_cores: int) -> np.ndarray:
    base, out_rows, in_rows = _band_layout(grid, n_cores)
    out = np.empty((grid, grid, 3), dtype=np.float32)
    for k in range(n_cores):
        ob = results[k]["oband"]  # [out_rows, 3, grid] f16
        take = out_rows - 1 if k < n_cores - 1 else out_rows
        out[base * k: base * k + take] = (
            ob[:take].transpose(0, 2, 1).astype(np.float32)
        )
    return out.reshape(grid * grid, 3)


def _run_stencil_on_device(vertices: np.ndarray, grid: int, n_cores: int,
                           trace: bool = False, repeats: int = 1, cfg=None):
    from concourse.bass_utils import run_bass_kernel_spmd

    in_maps = _make_in_maps(vertices, grid, n_cores)
    nc = _get_program(grid, n_cores, repeats, cfg)
    kres = run_bass_kernel_spmd(nc, in_maps, list(range(n_cores)), trace=trace)
    return _assemble_out(kres.results, grid, n_cores), kres


def kernel(vertices: np.ndarray, faces: np.ndarray) -> np.ndarray:
    vertices = np.asarray(vertices, dtype=np.float32)
    faces = np.asarray(faces)
    grid = int(round(np.sqrt(vertices.shape[0])))
    if (
        grid * grid == vertices.shape[0]
        and (grid - 1) % N_CORES == 0
        and _is_structured(faces, grid)
    ):
        out, _ = _run_stencil_on_device(vertices, grid, N_CORES)
        return out
    print("kernel: faces are not the structured triangulation; host fallback",
          file=sys.stderr)
    return _host_fallback(vertices, faces)



# revision 32
# speedup vs baseline: 1.0395x; 1.0395x over previous
"""Mesh vertex-normals kernel v3 for 8 TRN2 NeuronCores (Bass/Tile).

Structure (per core, on its row band of the padded vertex grid):
  * SoA layout: tiles are [rows, 3, cols] fp32 planes, so every
    elementwise op (including the 12 cross-product component mults) is
    unit-stride on the free axis.
  * Folded band: the 58-row leftover block is folded into column strips
    stacked on the partition axis (116 busy partitions instead of 58).
  * S-path (edges, cross products, T=C1+C2) computed in f32: any fp16
    rounding before the stencil sum blows up the ~70 vertices whose
    aggregate normal nearly cancels (|S| ~ 0.02) past the 2e-2 gate.
  * The vertex-normal stencil S = SH@t(c+1) + SH@c1 + I@t + I@c2(c+1)
    accumulates on the tensor engine into PSUM (SH = shift-down-one-
    partition matrix), eliminating the p/q/s adds from the vector
    engines.
  * Norm tail in fp16 (safe: rounding the final S is relative error):
    ACT Square from PSUM -> nsq adds -> ACT Sqrt(+eps) -> reciprocal ->
    packed fp16 multiply; output stored as fp16 planes, host converts.
  * Engine split is DVE-heavy: gpsimd (Pool) measures ~2.2 ns/elem on
    real HW vs DVE 1.04 (the v1 sim model's 0.833 for Pool is wrong).
  * Emission is software-pipelined: stage k of unit u emits at step
    u + k, so each in-order engine queue interleaves independent units
    (~2.6x faster than unit-sequential emission on HW).

Host side: pad (edge mode) + transpose to [rows, 3, cols] f32 planes;
output fp16 planes -> f32 [N, 3].
"""

import sys

sys.path.insert(0, "/opt/trn_rl_repo")

import numpy as np

GRID = 1449
N_CORES = 8


# ---------------------------------------------------------------------------
# host-side helpers
# ---------------------------------------------------------------------------

def _is_structured(faces: np.ndarray, grid: int) -> bool:
    n_quads = (grid - 1) * (grid - 1)
    if faces.shape != (2 * n_quads, 3):
        return False
    idx = np.arange(grid * grid, dtype=np.int64).reshape(grid, grid)
    i00 = idx[:-1, :-1].ravel()
    i01 = idx[:-1, 1:].ravel()
    i10 = idx[1:, :-1].ravel()
    i11 = idx[1:, 1:].ravel()
    f = faces
    return (
        np.array_equal(f[:n_quads, 0], i00)
        and np.array_equal(f[:n_quads, 1], i01)
        and np.array_equal(f[:n_quads, 2], i11)
        and np.array_equal(f[n_quads:, 0], i00)
        and np.array_equal(f[n_quads:, 1], i11)
        and np.array_equal(f[n_quads:, 2], i10)
    )


def _host_fallback(vertices: np.ndarray, faces: np.ndarray) -> np.ndarray:
    n_vertices = vertices.shape[0]
    va = vertices[faces[:, 0]]
    vb = vertices[faces[:, 1]]
    vc = vertices[faces[:, 2]]
    cross = np.cross(vb - va, vc - vb).astype(np.float32)
    norm = np.linalg.norm(cross, axis=-1, keepdims=True)
    weighted = (cross / norm) * (norm * 0.5)
    data = np.broadcast_to(weighted[:, None, :], (faces.shape[0], 3, 3)).reshape(-1, 3)
    summed = np.zeros((n_vertices, 3), dtype=np.float32)
    np.add.at(summed, faces.reshape(-1), data)
    norms = np.linalg.norm(summed, axis=-1, keepdims=True)
    return (summed / np.maximum(norms, 1e-10)).astype(np.float32)


def _band_layout(grid: int, n_cores: int):
    base = (grid - 1) // n_cores
    assert base * n_cores == grid - 1, "grid-1 must divide evenly"
    out_rows = base + 1
    in_rows = base + 3
    return base, out_rows, in_rows


def _col_chunks(width: int, chunk: int):
    return [(c0, min(chunk, width - c0)) for c0 in range(0, width, chunk)]


def _overlap_chunks(total: int, n: int):
    """n equal-width chunks covering [0, total); later chunks may overlap
    earlier ones. Yields (c0, so, wst): load cols c0..c0+w, store local
    cols so..so+wst to grid cols c0+so..c0+so+wst. All widths equal w."""
    w = -(-total // n)
    out = []
    for j in range(n):
        store_start = j * w
        store_end = min((j + 1) * w, total)
        c0 = min(j * w, total - w)
        out.append((c0, store_start - c0, store_end - store_start))
    return w, out


def _fold_units(grid: int, n_cores: int, chunks_a: int, chunks_b: int = 1):
    """Units: each = dict(P, w, rects=[(p0, nv, r0, c0, so, wst)]).

    Rect semantics: partitions p0..p0+nv hold padded-band v-rows
    r0..r0+nv; loads fetch w+2 cols from c0; stores write local cols
    so..so+wst to grid cols c0+so..
    """
    base, out_rows, in_rows = _band_layout(grid, n_cores)
    units = []
    if in_rows <= 128:
        w, chunks = _overlap_chunks(grid, chunks_a)
        for c0, so, wst in chunks:
            units.append(dict(P=in_rows, w=w,
                              rects=[(0, in_rows, 0, c0, so, wst)]))
        return units
    # 128-row rect A + leftover rect B folded into column strips
    nv_b = in_rows - 126
    assert nv_b >= 3
    w, chunks = _overlap_chunks(grid, chunks_a)
    for c0, so, wst in chunks:
        units.append(dict(P=128, w=w, rects=[(0, 128, 0, c0, so, wst)]))
    nstrips = 128 // nv_b
    wb, bstrips = _overlap_chunks(grid, nstrips)
    # chunk each strip's columns as well, so B tiles stay small
    wbc, bcols = _overlap_chunks(wb, chunks_b)
    for (coff, so2, wst2) in bcols:
        ch_lo, ch_hi = coff + so2, coff + so2 + wst2
        rects = []
        for j, (c0s, so, wst) in enumerate(bstrips):
            # intersect the strip's store range [so, so+wst) with the
            # column chunk's store range, both in strip-local coords
            lo = max(so, ch_lo)
            hi = min(so + wst, ch_hi)
            if hi <= lo:
                continue
            rects.append((j * nv_b, nv_b, 126, c0s + coff, lo - coff, hi - lo))
        units.append(dict(P=nstrips * nv_b, w=wbc, rects=rects))
    return units


# ---------------------------------------------------------------------------
# device program
# ---------------------------------------------------------------------------

DEFAULT_CFG = dict(
    chunks_a=5,
    chunks_b=3,
    sw_pipe=1,
    io_bufs=3,
    wk_bufs=4,
    psum_bufs=4,
    psum_cols=150,     # cols per PSUM chunk (x3 planes x4B <= 2KB)
    sq="act",          # 'act' | 'v' | 'g'
    o_bcast=True,
    o_s16=True,        # ACT-copy S from PSUM to fp16, packed multiply
    # vyf = (SH - I) @ v on the tensor engine: kills the vdn double-load
    # (8 fewer HBM DMAs/iter on the SP HWDGE ring, worth ~7 us/iter)
    vyf_pe=True,
    vyf_cp="act",
    pshift="pe",
    stencil="pe",      # S = SH@t(c+1) + SH@c1 + I@t + I@c2(c+1) in PSUM
    # engine per op: 'v' = vector (DVE), 'g' = gpsimd (Pool).
    # Pool measures ~2.2 ns/elem on HW vs DVE 1.04, so DVE-heavy.
    eng=dict(vyf="v", hx="v", dd="v", mm1="v", mm2="g", c1="g", c2="g",
             t="v", p="g", q="v", s="v", nsq="v", o="v"),
    mm_map="vgvgvgvgvgvv",
    st_eng="act",
    s_f16=True,
    # unroll the repeat loop body 8x: cross-iteration overlap through the
    # For_i boundary (per-iteration barrier tax amortized), ~15 us/iter
    unroll=8,
)


def _cfg_key(cfg):
    e = cfg["eng"]
    return (cfg["chunks_a"], cfg.get("chunks_b", 1),
            cfg["io_bufs"], cfg["wk_bufs"], cfg["sq"],
            cfg.get("o_bcast", True), cfg.get("vyf_pe", False),
            cfg.get("pshift", "dma"), cfg.get("psum_cols", 162),
            cfg.get("psum_bufs", 4), cfg.get("st_eng", "act"),
            cfg.get("s_f16", True), cfg.get("stencil", "dve"),
            cfg.get("mm_map"), cfg.get("o_s16", False),
            cfg.get("nsq_pe", False), cfg.get("fine_stages", False),
            cfg.get("norm_chunked", False),
            cfg.get("mm_dt", "f32"), cfg.get("rsqrt", False),
            cfg.get("sw_pipe", 1), cfg.get("ablate"),
            cfg.get("ld_v", "sp"), cfg.get("ld_vdn", "sp"),
            cfg.get("vyf_cp", "act"), cfg.get("st_chunked", False),
            cfg.get("unroll", 1),
            tuple(sorted(e.items())))


def _build_program(grid: int, n_cores: int, repeats: int = 1, cfg=None):
    import contextlib

    import concourse.bacc as bacc
    import concourse.tile as tile
    from concourse import mybir

    cfg = cfg or DEFAULT_CFG
    f16 = mybir.dt.float16
    f32 = mybir.dt.float32

    base, out_rows, in_rows = _band_layout(grid, n_cores)
    W = grid + 2

    nc = bacc.Bacc()
    vband = nc.dram_tensor("vband", [in_rows, 3, W], f32, kind="ExternalInput")

    units = _fold_units(grid, n_cores, cfg["chunks_a"], cfg.get("chunks_b", 1))
    for i, u in enumerate(units):
        u["idx"] = i

    if cfg.get("st_chunked", False):
        # chunked output: each (unit, rect) stores its full-width o tile
        # rows as a flat per-partition-contiguous slab; host reassembles
        off = 0
        for u in units:
            u["offs"] = []
            for (p0, nv, r0, c0, so, wst) in u["rects"]:
                u["offs"].append(off)
                off += (nv - 2) * 3 * u["w"]
        oband = nc.dram_tensor("obuf", [off], f16, kind="ExternalOutput")
    else:
        oband = nc.dram_tensor("oband", [out_rows, 3, grid], f16,
                               kind="ExternalOutput")

    with tile.TileContext(nc) as tc:
        with (
            tc.tile_pool(name="io", bufs=cfg["io_bufs"]) as io,
            tc.tile_pool(name="wk", bufs=cfg["wk_bufs"]) as wk,
            tc.tile_pool(name="ps", bufs=cfg.get("psum_bufs", 4),
                         space="PSUM") as psp,
            tc.tile_pool(name="cst", bufs=1) as cst,
        ):
            from concourse.masks import make_identity

            eps_tile = cst.tile([128, 1], f16, tag="eps")
            nc.vector.memset(eps_tile[:, :], 1e-7)
            # stencil matmul weights: float32r when mm_dt says so (the
            # verifier requires fp32r matmul inputs to be produced rounded)
            _pe_st = cfg.get("stencil", "dve") in ("pe", "pe2", "pe6", "pe12")
            wdt = (mybir.dt.bfloat16
                   if cfg.get("mm_dt", "f32") == "f32r" and _pe_st else f32)
            # tid[:, 1:129] = down-shift matrix SH[k, m] = 1 iff k == m+1
            tid = cst.tile([128, 130], wdt, tag="tid")
            nc.gpsimd.memset(tid[:, :], 0.0)
            make_identity(nc, tid[:, 0:128], nomemset=True)
            tid16 = None
            if cfg.get("nsq_pe", False):
                tid16 = cst.tile([128, 128], f16, tag="tid16")
                nc.gpsimd.memset(tid16[:, :], 0.0)
                make_identity(nc, tid16[:, :], nomemset=True)
            # tmix[:, 1:129][k, m] = +1 if k == m+1 else (-1 if k == m)
            tmix = cst.tile([128, 130], f32, tag="tmix")
            nc.gpsimd.memset(tmix[:, :], 0.0)
            make_identity(nc, tmix[:, 0:128], nomemset=True)
            nc.gpsimd.affine_select(
                out=tmix[:, 1:129], in_=tmix[:, 1:129],
                compare_op=mybir.AluOpType.not_equal, fill=-1.0, base=0,
                pattern=[[-1, 128]], channel_multiplier=1,
            )
            kvconst = None
            if cfg.get("ablate") == "crossnd":
                kvconst = cst.tile([128, 3, 512], f32, tag="kvconst")
                nc.gpsimd.memset(kvconst[:, :, :], 1.25)
            tneg = None
            if cfg.get("stencil") == "pe12":
                # tneg[:, 1:129] = -SH, tneg[:, 0:128] = -I
                tneg = cst.tile([128, 130], wdt, tag="tneg")
                nc.gpsimd.memset(tneg[:, :], 0.0)
                make_identity(nc, tneg[:, 0:128], nomemset=True)
                nc.vector.tensor_scalar_mul(out=tneg[:, :], in0=tneg[:, :],
                                            scalar1=-1.0)

            unroll = cfg.get("unroll", 1)
            n_iter = max(1, repeats // unroll)
            loop = tc.For_i(0, n_iter, 1) if repeats > 1 else contextlib.nullcontext()
            with loop:
                for _rep in range(unroll if repeats > 1 else 1):
                    stages = [
                        _emit_unit(nc, io, wk, psp, eps_tile, tid, tmix, unit,
                                   vband, oband, mybir, cfg, tid16, tneg,
                                   kvconst)
                        for unit in units
                    ]
                    skew = cfg.get("sw_pipe", 0)
                    if skew:
                        # software pipeline: stage k of unit u emits at step
                        # u + k*skew; later stages (older units) first, so
                        # each in-order engine queue interleaves units.
                        nst = len(stages[0])
                        total = len(units) + (nst - 1) * skew
                        for step in range(total):
                            for stg in range(nst - 1, -1, -1):
                                ui = step - stg * skew
                                if 0 <= ui < len(units):
                                    stages[ui][stg]()
                    else:
                        for fs in stages:
                            for f in fs:
                                f()

    nc.finalize()
    return nc


def _psum_chunks(width: int, chunk: int):
    return [(j0, min(chunk, width - j0)) for j0 in range(0, width, chunk)]


def _emit_unit(nc, io, wk, psp, eps_tile, tid, tmix, unit, vband, oband,
               mybir, cfg, tid16=None, tneg=None, kvconst=None):
    """Returns a list of stage closures: [load, vyf, crosses, stencil, norm].

    Calling them in order emits the unit; a software-pipelined caller can
    interleave stages of different units.
    """
    f16 = mybir.dt.float16
    f32 = mybir.dt.float32
    Alu = mybir.AluOpType
    Act = mybir.ActivationFunctionType
    ui = unit.get("idx", 0)
    is_a_unit = ui < cfg["chunks_a"]
    ENG = {"v": nc.vector, "g": nc.gpsimd,
           "a": nc.vector if ui % 2 == 0 else nc.gpsimd,
           "b": nc.gpsimd if ui % 2 == 0 else nc.vector,
           "u": nc.vector if is_a_unit else nc.gpsimd,
           "w": nc.gpsimd if is_a_unit else nc.vector}
    eng = {k: ENG[v] for k, v in cfg["eng"].items()}

    def tt(tag, out, in0, in1, op):
        eng[tag].tensor_tensor(out=out, in0=in0, in1=in1, op=op)

    P, w, rects = unit["P"], unit["w"], unit["rects"]
    w2 = w + 2
    in_rows = vband.shape[0]
    pcols = cfg.get("psum_cols", 162)
    ts = {}  # tiles shared across stages
    # ablation level for bottleneck experiments (output is wrong):
    # loads < edges < cross1 < cross < stencil < None (full)
    abl = cfg.get("ablate")
    ABL_ORD = {"loads": 0, "edges": 1, "cross1": 2, "cross": 3,
               "crossnd": 3, "stencil": 4, None: 5}
    alvl = ABL_ORD[abl]
    nodep = abl == "crossnd"

    def _emit_stores(o):
        st = {"sp": nc.sync, "act": nc.scalar,
              "g": nc.gpsimd}[cfg.get("st_eng", "act")]
        if cfg.get("st_chunked", False):
            for i, (p0, nv, r0, c0, so, wst) in enumerate(rects):
                ns = nv - 2
                off = unit["offs"][i]
                dst = oband[off:off + ns * 3 * w].rearrange(
                    "(r p c) -> r p c", r=ns, p=3, c=w)
                st.dma_start(out=dst, in_=o[p0:p0 + ns, :, 0:w])
        else:
            for (p0, nv, r0, c0, so, wst) in rects:
                ns = nv - 2
                st.dma_start(
                    out=oband[r0:r0 + ns, :, c0 + so:c0 + so + wst],
                    in_=o[p0:p0 + ns, :, so:so + wst])

    def _abl_store():
        # mimic the tail's ACT copy + store, from whatever was computed
        src = {0: lambda: ts["v"][:, :, 0:w], 1: lambda: ts["hx"][:, :, 0:w],
               2: lambda: ts["m1"][:, :, 0:w], 3: lambda: ts["m3"][:, :, 0:w],
               }[alvl]()
        o = io.tile([P, 3, w], f16, tag="o", name="o")
        nc.scalar.activation(out=o[:, :, :], in_=src, func=Act.Copy)
        _emit_stores(o)

    DMA_ENG = {"sp": nc.sync, "act": nc.scalar, "g": nc.gpsimd,
               "v": nc.vector}
    ld_v = DMA_ENG[cfg.get("ld_v", "sp")]
    ld_vdn = DMA_ENG[cfg.get("ld_vdn", "sp")]

    def stage_load():
        v = ts["v"] = io.tile([P, 3, w2], f32, tag="v", name="v")
        for (p0, nv, r0, c0, so, wst) in rects:
            ld_v.dma_start(out=v[p0:p0 + nv, :, :],
                           in_=vband[r0:r0 + nv, :, c0:c0 + w2])
        if not cfg.get("vyf_pe", False):
            vdn = ts["vdn"] = io.tile([P, 3, w2], f32, tag="vdn", name="vdn")
            for (p0, nv, r0, c0, so, wst) in rects:
                # duplicate the band's last row if the shifted window runs
                # off the end (that partition is never used)
                n_load = min(nv, in_rows - (r0 + 1))
                ld_vdn.dma_start(
                    out=vdn[p0:p0 + n_load, :, :],
                    in_=vband[r0 + 1:r0 + 1 + n_load, :, c0:c0 + w2])
                if n_load < nv:
                    ld_vdn.dma_start(
                        out=vdn[p0 + nv - 1:p0 + nv, :, :],
                        in_=vband[in_rows - 1:in_rows, :, c0:c0 + w2])

    def stage_vyf():
        if alvl < 1:
            return
        v = kvconst[:P, :, 0:w2] if nodep else ts["v"]
        vyf = ts["vyf"] = wk.tile([P, 3, w2], f32, tag="vyf", name="vyf")
        if nodep:
            vyf = ts["vyf"] = wk.tile([P, 3, w2], f32, tag="vyf", name="vyf")
            tt("vyf", vyf[:, :, :], kvconst[:P, :, 0:w2],
               kvconst[:P, :, 0:w2], Alu.subtract)
            hx = ts["hx"] = wk.tile([P, 3, w + 1], f32, tag="hx", name="hx")
            tt("hx", hx[:, :, :], kvconst[:P, :, 1:w2],
               kvconst[:P, :, 0:w + 1], Alu.subtract)
            return
        if cfg.get("vyf_pe", False):
            # vyf = (SH - I) @ v on the tensor engine; ACT copies PSUM out.
            # Seam partitions mix adjacent rects; they are never consumed.
            for j0, pw in _psum_chunks(w2, pcols):
                psv = psp.tile([128, 3, pw], f32, tag="psv", name="psv")
                nc.tensor.matmul(out=psv[:, :, :], lhsT=tmix[0:P, 1:129],
                                 rhs=v[:, :, j0:j0 + pw], start=True, stop=True)
                if cfg.get("vyf_cp", "act") == "act":
                    nc.scalar.activation(out=vyf[:, :, j0:j0 + pw],
                                         in_=psv[0:P, :, :], func=Act.Copy)
                else:
                    ENG[cfg["vyf_cp"]].tensor_scalar(
                        out=vyf[:, :, j0:j0 + pw], in0=psv[0:P, :, :],
                        scalar1=1.0, scalar2=None, op0=Alu.mult)
        else:
            tt("vyf", vyf[:, :, :], ts["vdn"][:, :, :], v[:, :, :],
               Alu.subtract)
        hx = ts["hx"] = wk.tile([P, 3, w + 1], f32, tag="hx", name="hx")
        tt("hx", hx[:, :, :], v[:, :, 1:w2], v[:, :, 0:w + 1], Alu.subtract)

    mm_map = cfg.get("mm_map")

    def mm(idx, dflt, out_, a, b):
        e = ENG[mm_map[idx]] if mm_map else eng[dflt]
        e.tensor_tensor(out=out_, in0=a, in1=b, op=Alu.mult)

    pe12 = cfg.get("stencil") == "pe12"
    # matmul-consumed tiles must be produced pre-rounded to float32r
    _pe_st = cfg.get("stencil", "dve") in ("pe", "pe2", "pe6", "pe12")
    mdt = (mybir.dt.float32r
           if cfg.get("mm_dt", "f32") == "f32r" and _pe_st else f32)
    m_mdt = mdt if pe12 else f32

    def stage_cross_a():
        if alvl < 2:
            return
        vyf, hx = ts["vyf"], ts["hx"]
        dd = ts["dd"] = wk.tile([P, 3, w + 1], f32, tag="dd", name="dd")
        if nodep:
            tt("dd", dd[:, :, :], kvconst[:P, :, 1:w2],
               kvconst[:P, :, 0:w + 1], Alu.subtract)
        elif "vdn" in ts:
            # dd = vdn(c+1) - v(c): straight from loads (one rounding,
            # no dependency on hx/vyf -> shorter critical chain)
            tt("dd", dd[:, :, :], ts["vdn"][:, :, 1:w2],
               ts["v"][:, :, 0:w + 1], Alu.subtract)
        else:
            tt("dd", dd[:, :, :], hx[:, :, :], vyf[:, :, 1:w2], Alu.add)
        m1 = ts["m1"] = wk.tile([P, 3, w + 1], m_mdt, tag="m1", name="m1")
        m2 = ts["m2"] = wk.tile([P, 3, w + 1], m_mdt, tag="m2", name="m2")
        for k in range(3):
            u, x = (k + 1) % 3, (k + 2) % 3
            mm(2 * k, "mm1", m1[:, k:k + 1, :], hx[:, u:u + 1, :],
               vyf[:, x:x + 1, 1:w2])
            mm(2 * k + 1, "mm2", m2[:, k:k + 1, :], hx[:, x:x + 1, :],
               vyf[:, u:u + 1, 1:w2])
        if not pe12:
            c1 = ts["c1"] = wk.tile([P, 3, w + 1], mdt, tag="c1", name="c1")
            tt("c1", c1[:, :, :], m1[:, :, :], m2[:, :, :], Alu.subtract)

    def stage_cross_b():
        if alvl < 3:
            return
        vyf, dd = ts["vyf"], ts["dd"]
        tag3, tag4 = ("m3", "m4") if pe12 else ("m1", "m2")
        m3 = ts["m3"] = wk.tile([P, 3, w + 1], m_mdt, tag=tag3, name="m3")
        m4 = ts["m4"] = wk.tile([P, 3, w + 1], m_mdt, tag=tag4, name="m4")
        for k in range(3):
            u, x = (k + 1) % 3, (k + 2) % 3
            mm(6 + 2 * k, "mm1", m3[:, k:k + 1, :], dd[:, u:u + 1, :],
               vyf[:, x:x + 1, 0:w + 1])
            mm(7 + 2 * k, "mm2", m4[:, k:k + 1, :], dd[:, x:x + 1, :],
               vyf[:, u:u + 1, 0:w + 1])
        if not pe12:
            c2 = ts["c2"] = wk.tile([P, 3, w + 1], mdt, tag="c2", name="c2")
            tt("c2", c2[:, :, :], m3[:, :, :], m4[:, :, :], Alu.subtract)

    def stage_cross():
        stage_cross_a()
        stage_cross_b()

    mm_dt = cfg.get("mm_dt", "f32")

    def _mm_cast(ap):
        if mm_dt == "f32r" and ap.dtype == f32:
            return ap.bitcast(mybir.dt.float32r)
        return ap

    def stage_stencil():
        if alvl < 4:
            return
        c1, c2 = ts.get("c1"), ts.get("c2")
        if cfg.get("stencil") == "pe6":
            # S = SH@c1(c+1) + SH@c2(c+1) + SH@c1 + I@c1 + I@c2 + I@c2(c+1)
            # (t = c1+c2 folded into the PE accumulation)
            ts["pss"] = []
            for j0, pw in _psum_chunks(w, pcols):
                pss = psp.tile([128, 3, pw], f32, tag="pss", name="pss")
                I, SH = tid[0:P, 0:128], tid[0:P, 1:129]
                terms = [
                    (SH, c1, 1), (SH, c2, 1), (SH, c1, 0),
                    (I, c1, 0), (I, c2, 0), (I, c2, 1),
                ]
                for i, (m, src, off) in enumerate(terms):
                    nc.tensor.matmul(
                        out=pss[:, :, :], lhsT=_mm_cast(m),
                        rhs=_mm_cast(src[:, :, off + j0:off + j0 + pw]),
                        start=(i == 0), stop=(i == len(terms) - 1))
                ts["pss"].append((j0, pw, pss))
            ts["Q"] = P
            return
        if cfg.get("stencil") == "pe12":
            # c1=m1-m2, c2=m3-m4, t=c1+c2 all folded into PE accumulation:
            # S = SH@(m1-m2+m3-m4)(c+1) + SH@(m1-m2)(c)
            #     + I@(m1-m2+m3-m4)(c) + I@(m3-m4)(c+1)
            m1, m2, m3, m4 = ts["m1"], ts["m2"], ts["m3"], ts["m4"]
            ts["pss"] = []
            for j0, pw in _psum_chunks(w, pcols):
                pss = psp.tile([128, 3, pw], f32, tag="pss", name="pss")
                I, SH = tid[0:P, 0:128], tid[0:P, 1:129]
                NI, NSH = tneg[0:P, 0:128], tneg[0:P, 1:129]
                terms = [
                    (SH, m1, 1), (NSH, m2, 1), (SH, m3, 1), (NSH, m4, 1),
                    (SH, m1, 0), (NSH, m2, 0),
                    (I, m1, 0), (NI, m2, 0), (I, m3, 0), (NI, m4, 0),
                    (I, m3, 1), (NI, m4, 1),
                ]
                for i, (m, src, off) in enumerate(terms):
                    nc.tensor.matmul(
                        out=pss[:, :, :], lhsT=_mm_cast(m),
                        rhs=_mm_cast(src[:, :, off + j0:off + j0 + pw]),
                        start=(i == 0), stop=(i == len(terms) - 1))
                ts["pss"].append((j0, pw, pss))
            ts["Q"] = P
            return
        # T = C1+C2; P = T(c+1)+C1; Q = T+C2(c+1); S = down(P)+Q
        t = wk.tile([P, 3, w + 1], mdt, tag="t", name="t")
        tt("t", t[:, :, :], c1[:, :, :], c2[:, :, :], Alu.add)
        if cfg.get("stencil") == "pe2":
            # p = t(c+1)+c1, q = t+c2(c+1) on DVE; S = SH@p + I@q on PE
            p = wk.tile([P, 3, w], mdt, tag="p", name="p")
            tt("p", p[:, :, :], t[:, :, 1:w + 1], c1[:, :, 0:w], Alu.add)
            q = wk.tile([P, 3, w], mdt, tag="q", name="q")
            tt("q", q[:, :, :], t[:, :, 0:w], c2[:, :, 1:w + 1], Alu.add)
            ts["pss"] = []
            for j0, pw in _psum_chunks(w, pcols):
                pss = psp.tile([128, 3, pw], f32, tag="pss", name="pss")
                I, SH = tid[0:P, 0:128], tid[0:P, 1:129]
                terms = [(SH, p, 0), (I, q, 0)]
                for i, (m, src, off) in enumerate(terms):
                    nc.tensor.matmul(
                        out=pss[:, :, :], lhsT=_mm_cast(m),
                        rhs=_mm_cast(src[:, :, off + j0:off + j0 + pw]),
                        start=(i == 0), stop=(i == len(terms) - 1))
                ts["pss"].append((j0, pw, pss))
            ts["Q"] = P
            return
        if cfg.get("stencil", "dve") == "pe":
            # S accumulates fully in PSUM:
            #   S = SH@t(c+1) + SH@c1(c) + I@t(c) + I@c2(c+1)
            ts["pss"] = []
            for j0, pw in _psum_chunks(w, pcols):
                pss = psp.tile([128, 3, pw], f32, tag="pss", name="pss")
                I, SH = tid[0:P, 0:128], tid[0:P, 1:129]
                terms = [(SH, t, 1), (SH, c1, 0), (I, t, 0), (I, c2, 1)]
                for i, (m, src, off) in enumerate(terms):
                    nc.tensor.matmul(
                        out=pss[:, :, :], lhsT=_mm_cast(m),
                        rhs=_mm_cast(src[:, :, off + j0:off + j0 + pw]),
                        start=(i == 0), stop=(i == len(terms) - 1))
                ts["pss"].append((j0, pw, pss))
            ts["Q"] = P
            return
        p = wk.tile([P, 3, w], f32, tag="dd", name="p")
        tt("p", p[:, :, :], t[:, :, 1:w + 1], c1[:, :, 0:w], Alu.add)
        q = wk.tile([P, 3, w], f32, tag="q", name="q")
        tt("q", q[:, :, :], t[:, :, 0:w], c2[:, :, 1:w + 1], Alu.add)

        sdt = f16 if cfg.get("s_f16", True) else f32
        s = ts["s"] = wk.tile([P, 3, w], sdt, tag="hx", name="s")
        if cfg.get("pshift", "dma") == "pe":
            # s = SH @ p + q: the shift runs on the tensor engine into PSUM
            for j0, pw in _psum_chunks(w, pcols):
                pss = psp.tile([128, 3, pw], f32, tag="pss", name="pss")
                nc.tensor.matmul(out=pss[:, :, :], lhsT=tid[0:P, 1:129],
                                 rhs=p[:, :, j0:j0 + pw], start=True,
                                 stop=True)
                tt("s", s[:, :, j0:j0 + pw], pss[0:P, :, :],
                   q[:, :, j0:j0 + pw], Alu.add)
            ts["Q"] = P
        else:
            # full-tile partition shift; seam partitions get cross-rect
            # garbage, which post-shift ops compute on but stores never read
            pdn = wk.tile([P, 3, w], f32, tag="vyf", name="pdn")
            nc.sync.dma_start(out=pdn[0:P - 1, :, :], in_=p[1:P, :, :])
            ts["Q"] = P - 1
            tt("s", s[0:P - 1, :, :], pdn[0:P - 1, :, :], q[0:P - 1, :, :],
               Alu.add)

    def stage_norm_chunked():
        # per-PSUM-chunk norm tail: sq/nsq/rn/o column-local, so each
        # chunk finishes (and stores) without waiting for the other
        Q = ts["Q"]
        o = io.tile([P, 3, w], f16, tag="o", name="o")
        for j0, pw, pss in ts["pss"]:
            sq = wk.tile([P, 3, pw], f16, tag="m1", name="sq")
            nc.scalar.activation(out=sq[0:Q, :, :], in_=pss[0:Q, :, :],
                                 func=Act.Square)
            nsq = wk.tile([P, 1, pw], f16, tag="nsq", name="nsq")
            tt("nsq", nsq[0:Q, :, :], sq[0:Q, 0:1, :], sq[0:Q, 1:2, :],
               Alu.add)
            tt("nsq", nsq[0:Q, :, :], nsq[0:Q, :, :], sq[0:Q, 2:3, :],
               Alu.add)
            rn = wk.tile([P, 1, pw], f16, tag="rn", name="rn")
            if cfg.get("rsqrt", False):
                nc.scalar.activation(out=rn[0:Q, :, :], in_=nsq[0:Q, :, :],
                                     func=Act.Rsqrt, bias=eps_tile[:Q, :])
            else:
                nc.scalar.activation(out=rn[0:Q, :, :], in_=nsq[0:Q, :, :],
                                     func=Act.Sqrt, bias=eps_tile[:Q, :])
                with nc.allow_low_precision(reason="1/norm fine in fp16"):
                    if cfg["eng"].get("rcp", "v") == "v":
                        nc.vector.reciprocal(out=rn[0:Q, :, :],
                                             in_=rn[0:Q, :, :])
                    else:
                        nc.gpsimd.reciprocal(out=rn[0:Q, :, :],
                                             in_=rn[0:Q, :, :])
            s16 = wk.tile([P, 3, pw], f16, tag="m2", name="s16")
            nc.scalar.activation(out=s16[0:Q, :, :], in_=pss[0:Q, :, :],
                                 func=Act.Copy)
            tt("o", o[0:Q, :, j0:j0 + pw], s16[0:Q, :, :],
               rn[0:Q, :, :].broadcast_to((Q, 3, pw)), Alu.mult)
        _emit_stores(o)

    def stage_norm():
        if alvl < 5:
            _abl_store()
            return
        if cfg.get("norm_chunked", False) and cfg.get("stencil") in ("pe", "pe2", "pe6", "pe12"):
            stage_norm_chunked()
            return
        Q = ts["Q"]
        sq = wk.tile([P, 3, w], f16, tag="m1", name="sq")
        if cfg.get("stencil", "dve") in ("pe", "pe2", "pe6", "pe12"):
            # S lives in PSUM chunks; square from PSUM, and o multiplies
            # the PSUM value directly.
            for j0, pw, pss in ts["pss"]:
                nc.scalar.activation(out=sq[0:Q, :, j0:j0 + pw],
                                     in_=pss[0:Q, :, :], func=Act.Square)
        elif cfg["sq"] == "act":
            nc.scalar.activation(out=sq[0:Q, :, :], in_=ts["s"][0:Q, :, :],
                                 func=Act.Square)
        else:
            ENG[cfg["sq"]].tensor_tensor(out=sq[0:Q, :, :],
                                         in0=ts["s"][0:Q, :, :],
                                         in1=ts["s"][0:Q, :, :], op=Alu.mult)
        rn = wk.tile([P, 1, w], f16, tag="rn", name="rn")
        if cfg.get("nsq_pe", False):
            # nsq = sq_x + sq_y + sq_z as 3 fp16 identity matmuls in PSUM
            psn = psp.tile([128, 1, w], f32, tag="psn", name="psn")
            for k in range(3):
                nc.tensor.matmul(out=psn[:, :, :], lhsT=tid16[0:P, :],
                                 rhs=sq[:, k:k + 1, :], start=(k == 0),
                                 stop=(k == 2))
            nc.scalar.activation(out=rn[0:Q, :, :], in_=psn[0:Q, :, :],
                                 func=Act.Sqrt, bias=eps_tile[:Q, :])
        else:
            nsq = wk.tile([P, 1, w], f16, tag="nsq", name="nsq")
            tt("nsq", nsq[0:Q, :, :], sq[0:Q, 0:1, :], sq[0:Q, 1:2, :],
               Alu.add)
            tt("nsq", nsq[0:Q, :, :], nsq[0:Q, :, :], sq[0:Q, 2:3, :],
               Alu.add)
            if cfg.get("rsqrt", False):
                nc.scalar.activation(out=rn[0:Q, :, :], in_=nsq[0:Q, :, :],
                                     func=Act.Rsqrt, bias=eps_tile[:Q, :])
            else:
                nc.scalar.activation(out=rn[0:Q, :, :], in_=nsq[0:Q, :, :],
                                     func=Act.Sqrt, bias=eps_tile[:Q, :])
        if not cfg.get("rsqrt", False):
            with nc.allow_low_precision(reason="1/norm fine in fp16"):
                if cfg["eng"].get("rcp", "v") == "v":
                    nc.vector.reciprocal(out=rn[0:Q, :, :], in_=rn[0:Q, :, :])
                else:
                    nc.gpsimd.reciprocal(out=rn[0:Q, :, :], in_=rn[0:Q, :, :])
        o = io.tile([P, 3, w], f16, tag="o", name="o")
        if cfg.get("stencil", "dve") in ("pe", "pe2", "pe6", "pe12"):
            if cfg.get("o_s16", False):
                s16 = wk.tile([P, 3, w], f16, tag="m2", name="s16")
                for j0, pw, pss in ts["pss"]:
                    nc.scalar.activation(out=s16[0:Q, :, j0:j0 + pw],
                                         in_=pss[0:Q, :, :], func=Act.Copy)
                tt("o", o[0:Q, :, :], s16[0:Q, :, :],
                   rn[0:Q, :, :].broadcast_to((Q, 3, w)), Alu.mult)
            else:
                for j0, pw, pss in ts["pss"]:
                    tt("o", o[0:Q, :, j0:j0 + pw], pss[0:Q, :, :],
                       rn[0:Q, :, j0:j0 + pw].broadcast_to((Q, 3, pw)),
                       Alu.mult)
        elif cfg.get("o_bcast", True):
            tt("o", o[0:Q, :, :], ts["s"][0:Q, :, :],
               rn[0:Q, :, :].broadcast_to((Q, 3, w)), Alu.mult)
        else:
            for k in range(3):
                tt("o", o[0:Q, k:k + 1, :], ts["s"][0:Q, k:k + 1, :],
                   rn[0:Q, :, :], Alu.mult)
        _emit_stores(o)

    if cfg.get("fine_stages", False):
        return [stage_load, stage_vyf, stage_cross_a, stage_cross_b,
                stage_stencil, stage_norm]
    return [stage_load, stage_vyf, stage_cross, stage_stencil, stage_norm]


_PROGRAM_CACHE: dict = {}


def _get_program(grid: int, n_cores: int, repeats: int = 1, cfg=None):
    cfg = cfg or DEFAULT_CFG
    key = (grid, n_cores, repeats, _cfg_key(cfg))
    if key not in _PROGRAM_CACHE:
        _PROGRAM_CACHE[key] = _build_program(grid, n_cores, repeats, cfg)
    return _PROGRAM_CACHE[key]


def _make_in_maps(vertices: np.ndarray, grid: int, n_cores: int):
    base, out_rows, in_rows = _band_layout(grid, n_cores)
    V = vertices.reshape(grid, grid, 3)
    VP = np.pad(V, ((1, 1), (1, 1), (0, 0)), mode="edge")
    VPT = np.ascontiguousarray(VP.transpose(0, 2, 1))
    return [
        {"vband": np.ascontiguousarray(VPT[base * k: base * k + in_rows])}
        for k in range(n_cores)
    ]


def _assemble_out(results, grid: int, n_cores: int, cfg=None) -> np.ndarray:
    cfg = cfg or DEFAULT_CFG
    base, out_rows, in_rows = _band_layout(grid, n_cores)
    out = np.empty((grid, grid, 3), dtype=np.float32)
    if cfg.get("st_chunked", False):
        units = _fold_units(grid, n_cores, cfg["chunks_a"],
                            cfg.get("chunks_b", 1))
        for k in range(n_cores):
            ob = results[k]["obuf"]
            off = 0
            for u in units:
                w = u["w"]
                for (p0, nv, r0, c0, so, wst) in u["rects"]:
                    ns = nv - 2
                    slab = ob[off:off + ns * 3 * w].reshape(ns, 3, w)
                    off += ns * 3 * w
                    rr = base * k + r0
                    out[rr:rr + ns, c0 + so:c0 + so + wst] = (
                        slab[:, :, so:so + wst].transpose(0, 2, 1)
                        .astype(np.float32))
        return out.reshape(grid * grid, 3)
    for k in range(n_cores):
        ob = results[k]["oband"]  # [out_rows, 3, grid] f16
        take = out_rows - 1 if k < n_cores - 1 else out_rows
        out[base * k: base * k + take] = (
            ob[:take].transpose(0, 2, 1).astype(np.float32)
        )
    return out.reshape(grid * grid, 3)


def _run_stencil_on_device(vertices: np.ndarray, grid: int, n_cores: int,
                           trace: bool = False, repeats: int = 1, cfg=None):
    from concourse.bass_utils import run_bass_kernel_spmd

    in_maps = _make_in_maps(vertices, grid, n_cores)
    nc = _get_program(grid, n_cores, repeats, cfg)
    kres = run_bass_kernel_spmd(nc, in_maps, list(range(n_cores)), trace=trace)
    return _assemble_out(kres.results, grid, n_cores, cfg), kres


def kernel(vertices: np.ndarray, faces: np.ndarray) -> np.ndarray:
    vertices = np.asarray(vertices, dtype=np.float32)
    faces = np.asarray(faces)
    grid = int(round(np.sqrt(vertices.shape[0])))
    if (
        grid * grid == vertices.shape[0]
        and (grid - 1) % N_CORES == 0
        and _is_structured(faces, grid)
    ):
        out, _ = _run_stencil_on_device(vertices, grid, N_CORES)
        return out
    print("kernel: faces are not the structured triangulation; host fallback",
          file=sys.stderr)
    return _host_fallback(vertices, faces)



# revision 33
# speedup vs baseline: 1.5618x; 1.5024x over previous
"""Mesh vertex-normals kernel v4 for 8 TRN2 NeuronCores (Bass/Tile).

Structure (per core, on its row band of the padded vertex grid):
  * SoA layout: tiles are [rows, 3, cols] fp32 planes, so every
    elementwise op (including the 12 cross-product component mults) is
    unit-stride on the free axis.
  * Folded band: the 58-row leftover block is folded into column strips
    stacked on the partition axis (116 busy partitions instead of 58).
  * S-path (edges, cross products, T=C1+C2) computed in f32: any fp16
    rounding before the stencil sum blows up the ~70 vertices whose
    aggregate normal nearly cancels (|S| ~ 0.02) past the 2e-2 gate.
    (float32r matmuls measure ~2^-11 input rounding on HW — same class
    as fp16 — so the fast PE mode is unusable here; stencil matmuls
    stay plain fp32 at 4 cycles/row.)
  * vyf = (SH - I) @ v on the tensor engine (vyf_pe): removes the vdn
    double-load, halving input HBM DMAs (8 fewer per iteration on the
    SP HWDGE ring, which serializes loads FIFO). dd = hx + vyf(c+1).
  * The vertex-normal stencil S = SH@t(c+1) + SH@c1 + I@t + I@c2(c+1)
    accumulates on the tensor engine into PSUM (SH = shift-down-one-
    partition matrix), eliminating the p/q/s adds from the vector
    engines.
  * Norm tail in fp16 (safe: rounding the final S is relative error):
    ACT Square from PSUM -> nsq adds -> ACT Sqrt(+eps) -> reciprocal ->
    packed fp16 multiply; output stored as fp16 planes, host converts.
  * Engine split is DVE-heavy: gpsimd (Pool) measures ~2.2 ns/elem on
    real HW vs DVE 1.04 (the v1 sim model's 0.833 for Pool is wrong).
  * Emission is software-pipelined: stage k of unit u emits at step
    u + k, so each in-order engine queue interleaves independent units
    (~2.6x faster than unit-sequential emission on HW).
  * The repeat loop body is unrolled 8x (cfg 'unroll'): per-iteration
    compute adds nearly 1:1 on top of the DMA floor regardless of data
    dependencies (measured via a loads-only ablation plus dependency-
    free compute), so widening the loop body amortizes that boundary
    and buys cross-iteration overlap (~15 us/iter at R=4001).

Host side: pad (edge mode) + transpose to [rows, 3, cols] f32 planes;
output fp16 planes -> f32 [N, 3].
"""

import sys

sys.path.insert(0, "/opt/trn_rl_repo")

import numpy as np

GRID = 1449
N_CORES = 8


# ---------------------------------------------------------------------------
# host-side helpers
# ---------------------------------------------------------------------------

def _is_structured(faces: np.ndarray, grid: int) -> bool:
    n_quads = (grid - 1) * (grid - 1)
    if faces.shape != (2 * n_quads, 3):
        return False
    idx = np.arange(grid * grid, dtype=np.int64).reshape(grid, grid)
    i00 = idx[:-1, :-1].ravel()
    i01 = idx[:-1, 1:].ravel()
    i10 = idx[1:, :-1].ravel()
    i11 = idx[1:, 1:].ravel()
    f = faces
    return (
        np.array_equal(f[:n_quads, 0], i00)
        and np.array_equal(f[:n_quads, 1], i01)
        and np.array_equal(f[:n_quads, 2], i11)
        and np.array_equal(f[n_quads:, 0], i00)
        and np.array_equal(f[n_quads:, 1], i11)
        and np.array_equal(f[n_quads:, 2], i10)
    )


def _host_fallback(vertices: np.ndarray, faces: np.ndarray) -> np.ndarray:
    n_vertices = vertices.shape[0]
    va = vertices[faces[:, 0]]
    vb = vertices[faces[:, 1]]
    vc = vertices[faces[:, 2]]
    cross = np.cross(vb - va, vc - vb).astype(np.float32)
    norm = np.linalg.norm(cross, axis=-1, keepdims=True)
    weighted = (cross / norm) * (norm * 0.5)
    data = np.broadcast_to(weighted[:, None, :], (faces.shape[0], 3, 3)).reshape(-1, 3)
    summed = np.zeros((n_vertices, 3), dtype=np.float32)
    np.add.at(summed, faces.reshape(-1), data)
    norms = np.linalg.norm(summed, axis=-1, keepdims=True)
    return (summed / np.maximum(norms, 1e-10)).astype(np.float32)


def _band_layout(grid: int, n_cores: int):
    base = (grid - 1) // n_cores
    assert base * n_cores == grid - 1, "grid-1 must divide evenly"
    out_rows = base + 1
    in_rows = base + 3
    return base, out_rows, in_rows


def _col_chunks(width: int, chunk: int):
    return [(c0, min(chunk, width - c0)) for c0 in range(0, width, chunk)]


def _overlap_chunks(total: int, n: int):
    """n equal-width chunks covering [0, total); later chunks may overlap
    earlier ones. Yields (c0, so, wst): load cols c0..c0+w, store local
    cols so..so+wst to grid cols c0+so..c0+so+wst. All widths equal w."""
    w = -(-total // n)
    out = []
    for j in range(n):
        store_start = j * w
        store_end = min((j + 1) * w, total)
        c0 = min(j * w, total - w)
        out.append((c0, store_start - c0, store_end - store_start))
    return w, out


def _fold_units(grid: int, n_cores: int, chunks_a: int, chunks_b: int = 1):
    """Units: each = dict(P, w, rects=[(p0, nv, r0, c0, so, wst)]).

    Rect semantics: partitions p0..p0+nv hold padded-band v-rows
    r0..r0+nv; loads fetch w+2 cols from c0; stores write local cols
    so..so+wst to grid cols c0+so..
    """
    base, out_rows, in_rows = _band_layout(grid, n_cores)
    units = []
    if in_rows <= 128:
        w, chunks = _overlap_chunks(grid, chunks_a)
        for c0, so, wst in chunks:
            units.append(dict(P=in_rows, w=w,
                              rects=[(0, in_rows, 0, c0, so, wst)]))
        return units
    # 128-row rect A + leftover rect B folded into column strips
    nv_b = in_rows - 126
    assert nv_b >= 3
    w, chunks = _overlap_chunks(grid, chunks_a)
    for c0, so, wst in chunks:
        units.append(dict(P=128, w=w, rects=[(0, 128, 0, c0, so, wst)]))
    nstrips = 128 // nv_b
    wb, bstrips = _overlap_chunks(grid, nstrips)
    # chunk each strip's columns as well, so B tiles stay small
    wbc, bcols = _overlap_chunks(wb, chunks_b)
    for (coff, so2, wst2) in bcols:
        ch_lo, ch_hi = coff + so2, coff + so2 + wst2
        rects = []
        for j, (c0s, so, wst) in enumerate(bstrips):
            # intersect the strip's store range [so, so+wst) with the
            # column chunk's store range, both in strip-local coords
            lo = max(so, ch_lo)
            hi = min(so + wst, ch_hi)
            if hi <= lo:
                continue
            rects.append((j * nv_b, nv_b, 126, c0s + coff, lo - coff, hi - lo))
        units.append(dict(P=nstrips * nv_b, w=wbc, rects=rects))
    return units


# ---------------------------------------------------------------------------
# device program
# ---------------------------------------------------------------------------

DEFAULT_CFG = dict(
    chunks_a=5,
    chunks_b=3,
    sw_pipe=1,
    io_bufs=3,
    wk_bufs=4,
    psum_bufs=4,
    psum_cols=150,     # cols per PSUM chunk (x3 planes x4B <= 2KB)
    sq="act",          # 'act' | 'v' | 'g'
    o_bcast=True,
    o_s16=True,        # ACT-copy S from PSUM to fp16, packed multiply
    # vyf = (SH - I) @ v on the tensor engine: kills the vdn double-load
    # (8 fewer HBM DMAs/iter on the SP HWDGE ring, worth ~7 us/iter)
    vyf_pe=True,
    vyf_cp="act",
    pshift="pe",
    stencil="pe",      # S = SH@t(c+1) + SH@c1 + I@t + I@c2(c+1) in PSUM
    # engine per op: 'v' = vector (DVE), 'g' = gpsimd (Pool).
    # Pool measures ~2.2 ns/elem on HW vs DVE 1.04, so DVE-heavy.
    eng=dict(vyf="v", hx="v", dd="v", mm1="v", mm2="g", c1="g", c2="g",
             t="v", p="g", q="v", s="v", nsq="v", o="v"),
    mm_map="vgvgvgvgvgvv",
    st_eng="act",
    s_f16=True,
    # unroll the repeat loop body 8x: cross-iteration overlap through the
    # For_i boundary (per-iteration barrier tax amortized), ~15 us/iter
    unroll=8,
)


def _cfg_key(cfg):
    e = cfg["eng"]
    return (cfg["chunks_a"], cfg.get("chunks_b", 1),
            cfg["io_bufs"], cfg["wk_bufs"], cfg["sq"],
            cfg.get("o_bcast", True), cfg.get("vyf_pe", False),
            cfg.get("pshift", "dma"), cfg.get("psum_cols", 162),
            cfg.get("psum_bufs", 4), cfg.get("st_eng", "act"),
            cfg.get("s_f16", True), cfg.get("stencil", "dve"),
            cfg.get("mm_map"), cfg.get("o_s16", False),
            cfg.get("nsq_pe", False), cfg.get("fine_stages", False),
            cfg.get("norm_chunked", False),
            cfg.get("mm_dt", "f32"), cfg.get("rsqrt", False),
            cfg.get("sw_pipe", 1), cfg.get("ablate"),
            cfg.get("ld_v", "sp"), cfg.get("ld_vdn", "sp"),
            cfg.get("vyf_cp", "act"), cfg.get("st_chunked", False),
            cfg.get("unroll", 1),
            tuple(sorted(e.items())))


def _build_program(grid: int, n_cores: int, repeats: int = 1, cfg=None):
    import contextlib

    import concourse.bacc as bacc
    import concourse.tile as tile
    from concourse import mybir

    cfg = cfg or DEFAULT_CFG
    f16 = mybir.dt.float16
    f32 = mybir.dt.float32

    base, out_rows, in_rows = _band_layout(grid, n_cores)
    W = grid + 2

    nc = bacc.Bacc()
    vband = nc.dram_tensor("vband", [in_rows, 3, W], f32, kind="ExternalInput")

    units = _fold_units(grid, n_cores, cfg["chunks_a"], cfg.get("chunks_b", 1))
    for i, u in enumerate(units):
        u["idx"] = i

    if cfg.get("st_chunked", False):
        # chunked output: each (unit, rect) stores its full-width o tile
        # rows as a flat per-partition-contiguous slab; host reassembles
        off = 0
        for u in units:
            u["offs"] = []
            for (p0, nv, r0, c0, so, wst) in u["rects"]:
                u["offs"].append(off)
                off += (nv - 2) * 3 * u["w"]
        oband = nc.dram_tensor("obuf", [off], f16, kind="ExternalOutput")
    else:
        oband = nc.dram_tensor("oband", [out_rows, 3, grid], f16,
                               kind="ExternalOutput")

    with tile.TileContext(nc) as tc:
        with (
            tc.tile_pool(name="io", bufs=cfg["io_bufs"]) as io,
            tc.tile_pool(name="wk", bufs=cfg["wk_bufs"]) as wk,
            tc.tile_pool(name="ps", bufs=cfg.get("psum_bufs", 4),
                         space="PSUM") as psp,
            tc.tile_pool(name="cst", bufs=1) as cst,
        ):
            from concourse.masks import make_identity

            eps_tile = cst.tile([128, 1], f16, tag="eps")
            nc.vector.memset(eps_tile[:, :], 1e-7)
            # stencil matmul weights: float32r when mm_dt says so (the
            # verifier requires fp32r matmul inputs to be produced rounded)
            _pe_st = cfg.get("stencil", "dve") in ("pe", "pe2", "pe6", "pe12")
            wdt = (mybir.dt.bfloat16
                   if cfg.get("mm_dt", "f32") == "f32r" and _pe_st else f32)
            # tid[:, 1:129] = down-shift matrix SH[k, m] = 1 iff k == m+1
            tid = cst.tile([128, 130], wdt, tag="tid")
            nc.gpsimd.memset(tid[:, :], 0.0)
            make_identity(nc, tid[:, 0:128], nomemset=True)
            tid16 = None
            if cfg.get("nsq_pe", False):
                tid16 = cst.tile([128, 128], f16, tag="tid16")
                nc.gpsimd.memset(tid16[:, :], 0.0)
                make_identity(nc, tid16[:, :], nomemset=True)
            # tmix[:, 1:129][k, m] = +1 if k == m+1 else (-1 if k == m)
            tmix = cst.tile([128, 130], f32, tag="tmix")
            nc.gpsimd.memset(tmix[:, :], 0.0)
            make_identity(nc, tmix[:, 0:128], nomemset=True)
            nc.gpsimd.affine_select(
                out=tmix[:, 1:129], in_=tmix[:, 1:129],
                compare_op=mybir.AluOpType.not_equal, fill=-1.0, base=0,
                pattern=[[-1, 128]], channel_multiplier=1,
            )
            kvconst = None
            if cfg.get("ablate") == "crossnd":
                kvconst = cst.tile([128, 3, 512], f32, tag="kvconst")
                nc.gpsimd.memset(kvconst[:, :, :], 1.25)
            tneg = None
            if cfg.get("stencil") == "pe12":
                # tneg[:, 1:129] = -SH, tneg[:, 0:128] = -I
                tneg = cst.tile([128, 130], wdt, tag="tneg")
                nc.gpsimd.memset(tneg[:, :], 0.0)
                make_identity(nc, tneg[:, 0:128], nomemset=True)
                nc.vector.tensor_scalar_mul(out=tneg[:, :], in0=tneg[:, :],
                                            scalar1=-1.0)

            unroll = cfg.get("unroll", 1)
            n_iter = max(1, repeats // unroll)
            loop = tc.For_i(0, n_iter, 1) if repeats > 1 else contextlib.nullcontext()
            with loop:
                for _rep in range(unroll if repeats > 1 else 1):
                    stages = [
                        _emit_unit(nc, io, wk, psp, eps_tile, tid, tmix, unit,
                                   vband, oband, mybir, cfg, tid16, tneg,
                                   kvconst)
                        for unit in units
                    ]
                    skew = cfg.get("sw_pipe", 0)
                    if skew:
                        # software pipeline: stage k of unit u emits at step
                        # u + k*skew; later stages (older units) first, so
                        # each in-order engine queue interleaves units.
                        nst = len(stages[0])
                        total = len(units) + (nst - 1) * skew
                        for step in range(total):
                            for stg in range(nst - 1, -1, -1):
                                ui = step - stg * skew
                                if 0 <= ui < len(units):
                                    stages[ui][stg]()
                    else:
                        for fs in stages:
                            for f in fs:
                                f()

    nc.finalize()
    return nc


def _psum_chunks(width: int, chunk: int):
    return [(j0, min(chunk, width - j0)) for j0 in range(0, width, chunk)]


def _emit_unit(nc, io, wk, psp, eps_tile, tid, tmix, unit, vband, oband,
               mybir, cfg, tid16=None, tneg=None, kvconst=None):
    """Returns a list of stage closures: [load, vyf, crosses, stencil, norm].

    Calling them in order emits the unit; a software-pipelined caller can
    interleave stages of different units.
    """
    f16 = mybir.dt.float16
    f32 = mybir.dt.float32
    Alu = mybir.AluOpType
    Act = mybir.ActivationFunctionType
    ui = unit.get("idx", 0)
    is_a_unit = ui < cfg["chunks_a"]
    ENG = {"v": nc.vector, "g": nc.gpsimd,
           "a": nc.vector if ui % 2 == 0 else nc.gpsimd,
           "b": nc.gpsimd if ui % 2 == 0 else nc.vector,
           "u": nc.vector if is_a_unit else nc.gpsimd,
           "w": nc.gpsimd if is_a_unit else nc.vector}
    eng = {k: ENG[v] for k, v in cfg["eng"].items()}

    def tt(tag, out, in0, in1, op):
        eng[tag].tensor_tensor(out=out, in0=in0, in1=in1, op=op)

    P, w, rects = unit["P"], unit["w"], unit["rects"]
    w2 = w + 2
    in_rows = vband.shape[0]
    pcols = cfg.get("psum_cols", 162)
    ts = {}  # tiles shared across stages
    # ablation level for bottleneck experiments (output is wrong):
    # loads < edges < cross1 < cross < stencil < None (full)
    abl = cfg.get("ablate")
    ABL_ORD = {"loads": 0, "edges": 1, "cross1": 2, "cross": 3,
               "crossnd": 3, "stencil": 4, None: 5}
    alvl = ABL_ORD[abl]
    nodep = abl == "crossnd"

    def _emit_stores(o):
        st = {"sp": nc.sync, "act": nc.scalar,
              "g": nc.gpsimd}[cfg.get("st_eng", "act")]
        if cfg.get("st_chunked", False):
            for i, (p0, nv, r0, c0, so, wst) in enumerate(rects):
                ns = nv - 2
                off = unit["offs"][i]
                dst = oband[off:off + ns * 3 * w].rearrange(
                    "(r p c) -> r p c", r=ns, p=3, c=w)
                st.dma_start(out=dst, in_=o[p0:p0 + ns, :, 0:w])
        else:
            for (p0, nv, r0, c0, so, wst) in rects:
                ns = nv - 2
                st.dma_start(
                    out=oband[r0:r0 + ns, :, c0 + so:c0 + so + wst],
                    in_=o[p0:p0 + ns, :, so:so + wst])

    def _abl_store():
        # mimic the tail's ACT copy + store, from whatever was computed
        src = {0: lambda: ts["v"][:, :, 0:w], 1: lambda: ts["hx"][:, :, 0:w],
               2: lambda: ts["m1"][:, :, 0:w], 3: lambda: ts["m3"][:, :, 0:w],
               }[alvl]()
        o = io.tile([P, 3, w], f16, tag="o", name="o")
        nc.scalar.activation(out=o[:, :, :], in_=src, func=Act.Copy)
        _emit_stores(o)

    DMA_ENG = {"sp": nc.sync, "act": nc.scalar, "g": nc.gpsimd,
               "v": nc.vector}
    ld_v = DMA_ENG[cfg.get("ld_v", "sp")]
    ld_vdn = DMA_ENG[cfg.get("ld_vdn", "sp")]

    def stage_load():
        v = ts["v"] = io.tile([P, 3, w2], f32, tag="v", name="v")
        for (p0, nv, r0, c0, so, wst) in rects:
            ld_v.dma_start(out=v[p0:p0 + nv, :, :],
                           in_=vband[r0:r0 + nv, :, c0:c0 + w2])
        if not cfg.get("vyf_pe", False):
            vdn = ts["vdn"] = io.tile([P, 3, w2], f32, tag="vdn", name="vdn")
            for (p0, nv, r0, c0, so, wst) in rects:
                # duplicate the band's last row if the shifted window runs
                # off the end (that partition is never used)
                n_load = min(nv, in_rows - (r0 + 1))
                ld_vdn.dma_start(
                    out=vdn[p0:p0 + n_load, :, :],
                    in_=vband[r0 + 1:r0 + 1 + n_load, :, c0:c0 + w2])
                if n_load < nv:
                    ld_vdn.dma_start(
                        out=vdn[p0 + nv - 1:p0 + nv, :, :],
                        in_=vband[in_rows - 1:in_rows, :, c0:c0 + w2])

    def stage_vyf():
        if alvl < 1:
            return
        v = kvconst[:P, :, 0:w2] if nodep else ts["v"]
        vyf = ts["vyf"] = wk.tile([P, 3, w2], f32, tag="vyf", name="vyf")
        if nodep:
            vyf = ts["vyf"] = wk.tile([P, 3, w2], f32, tag="vyf", name="vyf")
            tt("vyf", vyf[:, :, :], kvconst[:P, :, 0:w2],
               kvconst[:P, :, 0:w2], Alu.subtract)
            hx = ts["hx"] = wk.tile([P, 3, w + 1], f32, tag="hx", name="hx")
            tt("hx", hx[:, :, :], kvconst[:P, :, 1:w2],
               kvconst[:P, :, 0:w + 1], Alu.subtract)
            return
        if cfg.get("vyf_pe", False):
            # vyf = (SH - I) @ v on the tensor engine; ACT copies PSUM out.
            # Seam partitions mix adjacent rects; they are never consumed.
            for j0, pw in _psum_chunks(w2, pcols):
                psv = psp.tile([128, 3, pw], f32, tag="psv", name="psv")
                nc.tensor.matmul(out=psv[:, :, :], lhsT=tmix[0:P, 1:129],
                                 rhs=v[:, :, j0:j0 + pw], start=True, stop=True)
                if cfg.get("vyf_cp", "act") == "act":
                    nc.scalar.activation(out=vyf[:, :, j0:j0 + pw],
                                         in_=psv[0:P, :, :], func=Act.Copy)
                else:
                    ENG[cfg["vyf_cp"]].tensor_scalar(
                        out=vyf[:, :, j0:j0 + pw], in0=psv[0:P, :, :],
                        scalar1=1.0, scalar2=None, op0=Alu.mult)
        else:
            tt("vyf", vyf[:, :, :], ts["vdn"][:, :, :], v[:, :, :],
               Alu.subtract)
        hx = ts["hx"] = wk.tile([P, 3, w + 1], f32, tag="hx", name="hx")
        tt("hx", hx[:, :, :], v[:, :, 1:w2], v[:, :, 0:w + 1], Alu.subtract)

    mm_map = cfg.get("mm_map")

    def mm(idx, dflt, out_, a, b):
        e = ENG[mm_map[idx]] if mm_map else eng[dflt]
        e.tensor_tensor(out=out_, in0=a, in1=b, op=Alu.mult)

    pe12 = cfg.get("stencil") == "pe12"
    # matmul-consumed tiles must be produced pre-rounded to float32r
    _pe_st = cfg.get("stencil", "dve") in ("pe", "pe2", "pe6", "pe12")
    mdt = (mybir.dt.float32r
           if cfg.get("mm_dt", "f32") == "f32r" and _pe_st else f32)
    m_mdt = mdt if pe12 else f32

    def stage_cross_a():
        if alvl < 2:
            return
        vyf, hx = ts["vyf"], ts["hx"]
        dd = ts["dd"] = wk.tile([P, 3, w + 1], f32, tag="dd", name="dd")
        if nodep:
            tt("dd", dd[:, :, :], kvconst[:P, :, 1:w2],
               kvconst[:P, :, 0:w + 1], Alu.subtract)
        elif "vdn" in ts:
            # dd = vdn(c+1) - v(c): straight from loads (one rounding,
            # no dependency on hx/vyf -> shorter critical chain)
            tt("dd", dd[:, :, :], ts["vdn"][:, :, 1:w2],
               ts["v"][:, :, 0:w + 1], Alu.subtract)
        else:
            tt("dd", dd[:, :, :], hx[:, :, :], vyf[:, :, 1:w2], Alu.add)
        m1 = ts["m1"] = wk.tile([P, 3, w + 1], m_mdt, tag="m1", name="m1")
        m2 = ts["m2"] = wk.tile([P, 3, w + 1], m_mdt, tag="m2", name="m2")
        for k in range(3):
            u, x = (k + 1) % 3, (k + 2) % 3
            mm(2 * k, "mm1", m1[:, k:k + 1, :], hx[:, u:u + 1, :],
               vyf[:, x:x + 1, 1:w2])
            mm(2 * k + 1, "mm2", m2[:, k:k + 1, :], hx[:, x:x + 1, :],
               vyf[:, u:u + 1, 1:w2])
        if not pe12:
            c1 = ts["c1"] = wk.tile([P, 3, w + 1], mdt, tag="c1", name="c1")
            tt("c1", c1[:, :, :], m1[:, :, :], m2[:, :, :], Alu.subtract)

    def stage_cross_b():
        if alvl < 3:
            return
        vyf, dd = ts["vyf"], ts["dd"]
        tag3, tag4 = ("m3", "m4") if pe12 else ("m1", "m2")
        m3 = ts["m3"] = wk.tile([P, 3, w + 1], m_mdt, tag=tag3, name="m3")
        m4 = ts["m4"] = wk.tile([P, 3, w + 1], m_mdt, tag=tag4, name="m4")
        for k in range(3):
            u, x = (k + 1) % 3, (k + 2) % 3
            mm(6 + 2 * k, "mm1", m3[:, k:k + 1, :], dd[:, u:u + 1, :],
               vyf[:, x:x + 1, 0:w + 1])
            mm(7 + 2 * k, "mm2", m4[:, k:k + 1, :], dd[:, x:x + 1, :],
               vyf[:, u:u + 1, 0:w + 1])
        if not pe12:
            c2 = ts["c2"] = wk.tile([P, 3, w + 1], mdt, tag="c2", name="c2")
            tt("c2", c2[:, :, :], m3[:, :, :], m4[:, :, :], Alu.subtract)

    def stage_cross():
        stage_cross_a()
        stage_cross_b()

    mm_dt = cfg.get("mm_dt", "f32")

    def _mm_cast(ap):
        if mm_dt == "f32r" and ap.dtype == f32:
            return ap.bitcast(mybir.dt.float32r)
        return ap

    def stage_stencil():
        if alvl < 4:
            return
        c1, c2 = ts.get("c1"), ts.get("c2")
        if cfg.get("stencil") == "pe6":
            # S = SH@c1(c+1) + SH@c2(c+1) + SH@c1 + I@c1 + I@c2 + I@c2(c+1)
            # (t = c1+c2 folded into the PE accumulation)
            ts["pss"] = []
            for j0, pw in _psum_chunks(w, pcols):
                pss = psp.tile([128, 3, pw], f32, tag="pss", name="pss")
                I, SH = tid[0:P, 0:128], tid[0:P, 1:129]
                terms = [
                    (SH, c1, 1), (SH, c2, 1), (SH, c1, 0),
                    (I, c1, 0), (I, c2, 0), (I, c2, 1),
                ]
                for i, (m, src, off) in enumerate(terms):
                    nc.tensor.matmul(
                        out=pss[:, :, :], lhsT=_mm_cast(m),
                        rhs=_mm_cast(src[:, :, off + j0:off + j0 + pw]),
                        start=(i == 0), stop=(i == len(terms) - 1))
                ts["pss"].append((j0, pw, pss))
            ts["Q"] = P
            return
        if cfg.get("stencil") == "pe12":
            # c1=m1-m2, c2=m3-m4, t=c1+c2 all folded into PE accumulation:
            # S = SH@(m1-m2+m3-m4)(c+1) + SH@(m1-m2)(c)
            #     + I@(m1-m2+m3-m4)(c) + I@(m3-m4)(c+1)
            m1, m2, m3, m4 = ts["m1"], ts["m2"], ts["m3"], ts["m4"]
            ts["pss"] = []
            for j0, pw in _psum_chunks(w, pcols):
                pss = psp.tile([128, 3, pw], f32, tag="pss", name="pss")
                I, SH = tid[0:P, 0:128], tid[0:P, 1:129]
                NI, NSH = tneg[0:P, 0:128], tneg[0:P, 1:129]
                terms = [
                    (SH, m1, 1), (NSH, m2, 1), (SH, m3, 1), (NSH, m4, 1),
                    (SH, m1, 0), (NSH, m2, 0),
                    (I, m1, 0), (NI, m2, 0), (I, m3, 0), (NI, m4, 0),
                    (I, m3, 1), (NI, m4, 1),
                ]
                for i, (m, src, off) in enumerate(terms):
                    nc.tensor.matmul(
                        out=pss[:, :, :], lhsT=_mm_cast(m),
                        rhs=_mm_cast(src[:, :, off + j0:off + j0 + pw]),
                        start=(i == 0), stop=(i == len(terms) - 1))
                ts["pss"].append((j0, pw, pss))
            ts["Q"] = P
            return
        # T = C1+C2; P = T(c+1)+C1; Q = T+C2(c+1); S = down(P)+Q
        t = wk.tile([P, 3, w + 1], mdt, tag="t", name="t")
        tt("t", t[:, :, :], c1[:, :, :], c2[:, :, :], Alu.add)
        if cfg.get("stencil") == "pe2":
            # p = t(c+1)+c1, q = t+c2(c+1) on DVE; S = SH@p + I@q on PE
            p = wk.tile([P, 3, w], mdt, tag="p", name="p")
            tt("p", p[:, :, :], t[:, :, 1:w + 1], c1[:, :, 0:w], Alu.add)
            q = wk.tile([P, 3, w], mdt, tag="q", name="q")
            tt("q", q[:, :, :], t[:, :, 0:w], c2[:, :, 1:w + 1], Alu.add)
            ts["pss"] = []
            for j0, pw in _psum_chunks(w, pcols):
                pss = psp.tile([128, 3, pw], f32, tag="pss", name="pss")
                I, SH = tid[0:P, 0:128], tid[0:P, 1:129]
                terms = [(SH, p, 0), (I, q, 0)]
                for i, (m, src, off) in enumerate(terms):
                    nc.tensor.matmul(
                        out=pss[:, :, :], lhsT=_mm_cast(m),
                        rhs=_mm_cast(src[:, :, off + j0:off + j0 + pw]),
                        start=(i == 0), stop=(i == len(terms) - 1))
                ts["pss"].append((j0, pw, pss))
            ts["Q"] = P
            return
        if cfg.get("stencil", "dve") == "pe":
            # S accumulates fully in PSUM:
            #   S = SH@t(c+1) + SH@c1(c) + I@t(c) + I@c2(c+1)
            ts["pss"] = []
            for j0, pw in _psum_chunks(w, pcols):
                pss = psp.tile([128, 3, pw], f32, tag="pss", name="pss")
                I, SH = tid[0:P, 0:128], tid[0:P, 1:129]
                terms = [(SH, t, 1), (SH, c1, 0), (I, t, 0), (I, c2, 1)]
                for i, (m, src, off) in enumerate(terms):
                    nc.tensor.matmul(
                        out=pss[:, :, :], lhsT=_mm_cast(m),
                        rhs=_mm_cast(src[:, :, off + j0:off + j0 + pw]),
                        start=(i == 0), stop=(i == len(terms) - 1))
                ts["pss"].append((j0, pw, pss))
            ts["Q"] = P
            return
        p = wk.tile([P, 3, w], f32, tag="dd", name="p")
        tt("p", p[:, :, :], t[:, :, 1:w + 1], c1[:, :, 0:w], Alu.add)
        q = wk.tile([P, 3, w], f32, tag="q", name="q")
        tt("q", q[:, :, :], t[:, :, 0:w], c2[:, :, 1:w + 1], Alu.add)

        sdt = f16 if cfg.get("s_f16", True) else f32
        s = ts["s"] = wk.tile([P, 3, w], sdt, tag="hx", name="s")
        if cfg.get("pshift", "dma") == "pe":
            # s = SH @ p + q: the shift runs on the tensor engine into PSUM
            for j0, pw in _psum_chunks(w, pcols):
                pss = psp.tile([128, 3, pw], f32, tag="pss", name="pss")
                nc.tensor.matmul(out=pss[:, :, :], lhsT=tid[0:P, 1:129],
                                 rhs=p[:, :, j0:j0 + pw], start=True,
                                 stop=True)
                tt("s", s[:, :, j0:j0 + pw], pss[0:P, :, :],
                   q[:, :, j0:j0 + pw], Alu.add)
            ts["Q"] = P
        else:
            # full-tile partition shift; seam partitions get cross-rect
            # garbage, which post-shift ops compute on but stores never read
            pdn = wk.tile([P, 3, w], f32, tag="vyf", name="pdn")
            nc.sync.dma_start(out=pdn[0:P - 1, :, :], in_=p[1:P, :, :])
            ts["Q"] = P - 1
            tt("s", s[0:P - 1, :, :], pdn[0:P - 1, :, :], q[0:P - 1, :, :],
               Alu.add)

    def stage_norm_chunked():
        # per-PSUM-chunk norm tail: sq/nsq/rn/o column-local, so each
        # chunk finishes (and stores) without waiting for the other
        Q = ts["Q"]
        o = io.tile([P, 3, w], f16, tag="o", name="o")
        for j0, pw, pss in ts["pss"]:
            sq = wk.tile([P, 3, pw], f16, tag="m1", name="sq")
            nc.scalar.activation(out=sq[0:Q, :, :], in_=pss[0:Q, :, :],
                                 func=Act.Square)
            nsq = wk.tile([P, 1, pw], f16, tag="nsq", name="nsq")
            tt("nsq", nsq[0:Q, :, :], sq[0:Q, 0:1, :], sq[0:Q, 1:2, :],
               Alu.add)
            tt("nsq", nsq[0:Q, :, :], nsq[0:Q, :, :], sq[0:Q, 2:3, :],
               Alu.add)
            rn = wk.tile([P, 1, pw], f16, tag="rn", name="rn")
            if cfg.get("rsqrt", False):
                nc.scalar.activation(out=rn[0:Q, :, :], in_=nsq[0:Q, :, :],
                                     func=Act.Rsqrt, bias=eps_tile[:Q, :])
            else:
                nc.scalar.activation(out=rn[0:Q, :, :], in_=nsq[0:Q, :, :],
                                     func=Act.Sqrt, bias=eps_tile[:Q, :])
                with nc.allow_low_precision(reason="1/norm fine in fp16"):
                    if cfg["eng"].get("rcp", "v") == "v":
                        nc.vector.reciprocal(out=rn[0:Q, :, :],
                                             in_=rn[0:Q, :, :])
                    else:
                        nc.gpsimd.reciprocal(out=rn[0:Q, :, :],
                                             in_=rn[0:Q, :, :])
            s16 = wk.tile([P, 3, pw], f16, tag="m2", name="s16")
            nc.scalar.activation(out=s16[0:Q, :, :], in_=pss[0:Q, :, :],
                                 func=Act.Copy)
            tt("o", o[0:Q, :, j0:j0 + pw], s16[0:Q, :, :],
               rn[0:Q, :, :].broadcast_to((Q, 3, pw)), Alu.mult)
        _emit_stores(o)

    def stage_norm():
        if alvl < 5:
            _abl_store()
            return
        if cfg.get("norm_chunked", False) and cfg.get("stencil") in ("pe", "pe2", "pe6", "pe12"):
            stage_norm_chunked()
            return
        Q = ts["Q"]
        sq = wk.tile([P, 3, w], f16, tag="m1", name="sq")
        if cfg.get("stencil", "dve") in ("pe", "pe2", "pe6", "pe12"):
            # S lives in PSUM chunks; square from PSUM, and o multiplies
            # the PSUM value directly.
            for j0, pw, pss in ts["pss"]:
                nc.scalar.activation(out=sq[0:Q, :, j0:j0 + pw],
                                     in_=pss[0:Q, :, :], func=Act.Square)
        elif cfg["sq"] == "act":
            nc.scalar.activation(out=sq[0:Q, :, :], in_=ts["s"][0:Q, :, :],
                                 func=Act.Square)
        else:
            ENG[cfg["sq"]].tensor_tensor(out=sq[0:Q, :, :],
                                         in0=ts["s"][0:Q, :, :],
                                         in1=ts["s"][0:Q, :, :], op=Alu.mult)
        rn = wk.tile([P, 1, w], f16, tag="rn", name="rn")
        if cfg.get("nsq_pe", False):
            # nsq = sq_x + sq_y + sq_z as 3 fp16 identity matmuls in PSUM
            psn = psp.tile([128, 1, w], f32, tag="psn", name="psn")
            for k in range(3):
                nc.tensor.matmul(out=psn[:, :, :], lhsT=tid16[0:P, :],
                                 rhs=sq[:, k:k + 1, :], start=(k == 0),
                                 stop=(k == 2))
            nc.scalar.activation(out=rn[0:Q, :, :], in_=psn[0:Q, :, :],
                                 func=Act.Sqrt, bias=eps_tile[:Q, :])
        else:
            nsq = wk.tile([P, 1, w], f16, tag="nsq", name="nsq")
            tt("nsq", nsq[0:Q, :, :], sq[0:Q, 0:1, :], sq[0:Q, 1:2, :],
               Alu.add)
            tt("nsq", nsq[0:Q, :, :], nsq[0:Q, :, :], sq[0:Q, 2:3, :],
               Alu.add)
            if cfg.get("rsqrt", False):
                nc.scalar.activation(out=rn[0:Q, :, :], in_=nsq[0:Q, :, :],
                                     func=Act.Rsqrt, bias=eps_tile[:Q, :])
            else:
                nc.scalar.activation(out=rn[0:Q, :, :], in_=nsq[0:Q, :, :],
                                     func=Act.Sqrt, bias=eps_tile[:Q, :])
        if not cfg.get("rsqrt", False):
            with nc.allow_low_precision(reason="1/norm fine in fp16"):
                if cfg["eng"].get("rcp", "v") == "v":
                    nc.vector.reciprocal(out=rn[0:Q, :, :], in_=rn[0:Q, :, :])
                else:
                    nc.gpsimd.reciprocal(out=rn[0:Q, :, :], in_=rn[0:Q, :, :])
        o = io.tile([P, 3, w], f16, tag="o", name="o")
        if cfg.get("stencil", "dve") in ("pe", "pe2", "pe6", "pe12"):
            if cfg.get("o_s16", False):
                s16 = wk.tile([P, 3, w], f16, tag="m2", name="s16")
                for j0, pw, pss in ts["pss"]:
                    nc.scalar.activation(out=s16[0:Q, :, j0:j0 + pw],
                                         in_=pss[0:Q, :, :], func=Act.Copy)
                tt("o", o[0:Q, :, :], s16[0:Q, :, :],
                   rn[0:Q, :, :].broadcast_to((Q, 3, w)), Alu.mult)
            else:
                for j0, pw, pss in ts["pss"]:
                    tt("o", o[0:Q, :, j0:j0 + pw], pss[0:Q, :, :],
                       rn[0:Q, :, j0:j0 + pw].broadcast_to((Q, 3, pw)),
                       Alu.mult)
        elif cfg.get("o_bcast", True):
            tt("o", o[0:Q, :, :], ts["s"][0:Q, :, :],
               rn[0:Q, :, :].broadcast_to((Q, 3, w)), Alu.mult)
        else:
            for k in range(3):
                tt("o", o[0:Q, k:k + 1, :], ts["s"][0:Q, k:k + 1, :],
                   rn[0:Q, :, :], Alu.mult)
        _emit_stores(o)

    if cfg.get("fine_stages", False):
        return [stage_load, stage_vyf, stage_cross_a, stage_cross_b,
                stage_stencil, stage_norm]
    return [stage_load, stage_vyf, stage_cross, stage_stencil, stage_norm]


_PROGRAM_CACHE: dict = {}


def _get_program(grid: int, n_cores: int, repeats: int = 1, cfg=None):
    cfg = cfg or DEFAULT_CFG
    key = (grid, n_cores, repeats, _cfg_key(cfg))
    if key not in _PROGRAM_CACHE:
        _PROGRAM_CACHE[key] = _build_program(grid, n_cores, repeats, cfg)
    return _PROGRAM_CACHE[key]


def _make_in_maps(vertices: np.ndarray, grid: int, n_cores: int):
    base, out_rows, in_rows = _band_layout(grid, n_cores)
    V = vertices.reshape(grid, grid, 3)
    VP = np.pad(V, ((1, 1), (1, 1), (0, 0)), mode="edge")
    VPT = np.ascontiguousarray(VP.transpose(0, 2, 1))
    return [
        {"vband": np.ascontiguousarray(VPT[base * k: base * k + in_rows])}
        for k in range(n_cores)
    ]


def _assemble_out(results, grid: int, n_cores: int, cfg=None) -> np.ndarray:
    cfg = cfg or DEFAULT_CFG
    base, out_rows, in_rows = _band_layout(grid, n_cores)
    out = np.empty((grid, grid, 3), dtype=np.float32)
    if cfg.get("st_chunked", False):
        units = _fold_units(grid, n_cores, cfg["chunks_a"],
                            cfg.get("chunks_b", 1))
        for k in range(n_cores):
            ob = results[k]["obuf"]
            off = 0
            for u in units:
                w = u["w"]
                for (p0, nv, r0, c0, so, wst) in u["rects"]:
                    ns = nv - 2
                    slab = ob[off:off + ns * 3 * w].reshape(ns, 3, w)
                    off += ns * 3 * w
                    rr = base * k + r0
                    out[rr:rr + ns, c0 + so:c0 + so + wst] = (
                        slab[:, :, so:so + wst].transpose(0, 2, 1)
                        .astype(np.float32))
        return out.reshape(grid * grid, 3)
    for k in range(n_cores):
        ob = results[k]["oband"]  # [out_rows, 3, grid] f16
        take = out_rows - 1 if k < n_cores - 1 else out_rows
        out[base * k: base * k + take] = (
            ob[:take].transpose(0, 2, 1).astype(np.float32)
        )
    return out.reshape(grid * grid, 3)


def _run_stencil_on_device(vertices: np.ndarray, grid: int, n_cores: int,
                           trace: bool = False, repeats: int = 1, cfg=None):
    from concourse.bass_utils import run_bass_kernel_spmd

    in_maps = _make_in_maps(vertices, grid, n_cores)
    nc = _get_program(grid, n_cores, repeats, cfg)
    kres = run_bass_kernel_spmd(nc, in_maps, list(range(n_cores)), trace=trace)
    return _assemble_out(kres.results, grid, n_cores, cfg), kres


def kernel(vertices: np.ndarray, faces: np.ndarray) -> np.ndarray:
    vertices = np.asarray(vertices, dtype=np.float32)
    faces = np.asarray(faces)
    grid = int(round(np.sqrt(vertices.shape[0])))
    if (
        grid * grid == vertices.shape[0]
        and (grid - 1) % N_CORES == 0
        and _is_structured(faces, grid)
    ):
        out, _ = _run_stencil_on_device(vertices, grid, N_CORES)
        return out
    print("kernel: faces are not the structured triangulation; host fallback",
          file=sys.stderr)
    return _host_fallback(vertices, faces)

